# revision 26
# baseline (speedup 1.0000x reference)
"""Trainium2 Bass kernel for nn_MultiHeadAttention (B=2, S=2048, D=1024, H=16, dk=dv=64).

Sharding: 8 cores = 2 batch groups x 4 cores. Core c (g=c//4, p=c%4) computes
attention for 4 heads (heads p*4..p*4+3) of batch g over all 2048 tokens, then
an 8-rank AllToAll redistributes the attention output so core c holds all 16
heads for its 512-token slice; fc + residual + LayerNorm run token-parallel.

Layout choices (all matmul contractions live on the partition axis):
  - host supplies q/k/v transposed (XT[c, i]) so projections need no on-device
    transpose: QhT/KhT = W.T-free (lhsT=W tiles), Vh token-major (lhsT=XT tiles)
  - scores are computed transposed, ST[j, i] (keys on partitions), two heads at
    a time via row-tiled K=64 matmuls; exp runs on ACT with scale=1/sqrt(dk)
  - attention output accumulates as OT[d, i] via col-tiled M=64 matmuls
    (lhsT = Vh), softmax denominators ride along as col-tiled M=1 ones-matmuls
  - normalization: r=1/sums broadcast across partitions with a K=1 ones-matmul,
    then one DVE multiply
  - fc contracts over an extended 2048-row axis (both groups' AllToAll blocks);
    the host zeroes the other group's rows of w_fc so no per-rank addressing is
    needed (the program is identical on all 8 cores).
"""

import os
import sys

import numpy as np

if "/opt/trn_rl_repo" not in sys.path:
    sys.path.insert(0, "/opt/trn_rl_repo")

B, S, D = 2, 2048, 1024
H, DK, DV = 16, 64, 64
LN_EPS = 1e-5

NCORES = 8
PG = 4          # cores per batch group
HPC = 4         # heads per core
DPC = HPC * DV  # 256 local output dims per core
SL = S // PG    # 512 tokens per core for fc/LN
NC_TILES = D // 128  # 8 contraction tiles for projections

_CACHE = {}


def _build(trivial_ln: bool, debug: bool = False):
    import concourse.bass as bass  # noqa: F401
    import concourse.mybir as mybir
    import concourse.tile as tile
    from concourse import bacc

    f32 = mybir.dt.float32
    AF = mybir.ActivationFunctionType
    OP = mybir.AluOpType

    nc = bacc.Bacc()

    xt_q = nc.dram_tensor("xt_q", [D, S], f32, kind="ExternalInput")
    xt_k = nc.dram_tensor("xt_k", [D, S], f32, kind="ExternalInput")
    xt_v = nc.dram_tensor("xt_v", [D, S], f32, kind="ExternalInput")
    wq_d = nc.dram_tensor("wq", [D, DPC], f32, kind="ExternalInput")
    wk_d = nc.dram_tensor("wk", [D, DPC], f32, kind="ExternalInput")
    wv_d = nc.dram_tensor("wv", [D, DPC], f32, kind="ExternalInput")
    wfcx_d = nc.dram_tensor("wfcx", [2 * D, D], f32, kind="ExternalInput")
    resid_d = nc.dram_tensor("resid", [SL, D], f32, kind="ExternalInput")
    gamma_d = nc.dram_tensor("gamma", [1, D], f32, kind="ExternalInput")
    beta_d = nc.dram_tensor("beta", [1, D], f32, kind="ExternalInput")
    out_d = nc.dram_tensor("out", [SL, D], f32, kind="ExternalOutput")
    dbg = {}
    if debug:
        dbg["qht"] = nc.dram_tensor("dbg_qht", [2, 128, S], f32, kind="ExternalOutput")
        dbg["kht"] = nc.dram_tensor("dbg_kht", [2, 128, S], f32, kind="ExternalOutput")
        dbg["vh"] = nc.dram_tensor("dbg_vh", [16, 128, DPC], f32, kind="ExternalOutput")
        dbg["ex"] = nc.dram_tensor("dbg_ex", [128, 1024], f32, kind="ExternalOutput")
        dbg["ot"] = nc.dram_tensor("dbg_ot", [128, 512], f32, kind="ExternalOutput")
        dbg["sums"] = nc.dram_tensor("dbg_sums", [1, 1024], f32, kind="ExternalOutput")
        dbg["osc"] = nc.dram_tensor("dbg_osc", [128, 512], f32, kind="ExternalOutput")
        dbg["ogout"] = nc.dram_tensor(
            "dbg_ogout", [NCORES, DPC, SL], f32, kind="ExternalOutput"
        )
        dbg["r"] = nc.dram_tensor("dbg_r", [1, 1024], f32, kind="ExternalOutput")
        dbg["bc"] = nc.dram_tensor("dbg_bc", [128, 512], f32, kind="ExternalOutput")

    with tile.TileContext(nc) as tc:
        with (
            tc.tile_pool(name="consts", bufs=1) as consts,
            tc.tile_pool(name="persist", bufs=1) as persist,
            tc.tile_pool(name="stream", bufs=3) as stream,
            tc.tile_pool(name="work", bufs=3) as work,
            tc.tile_pool(name="dram", bufs=1, space="DRAM") as dram,
        ):
            ones_sb = consts.tile([128, 128], f32, tag="ones", name="ones_sb")
            nc.vector.memset(ones_sb[:], 1.0)
            eps_sb = consts.tile([128, 1], f32, tag="eps", name="eps_sb")
            nc.vector.memset(eps_sb[:], LN_EPS)

            # ---- weights for the projections
            wq_sb, wk_sb, wv_sb = [], [], []
            for ci in range(NC_TILES):
                for lst, dsrc, nm in (
                    (wq_sb, wq_d, "wq"),
                    (wk_sb, wk_d, "wk"),
                    (wv_sb, wv_d, "wv"),
                ):
                    t = persist.tile([128, DPC], f32, tag=f"{nm}{ci}", name=f"{nm}{ci}_sb")
                    nc.sync.dma_start(out=t[:], in_=dsrc[ci * 128 : (ci + 1) * 128, :])
                    lst.append(t)

            # ---- gamma/beta broadcast tiles (general LN path only)
            gbc_sb = bbc_sb = None
            if not trivial_ln:
                gam_row = consts.tile([1, D], f32, tag="gam_row", name="gam_row")
                nc.sync.dma_start(out=gam_row[:], in_=gamma_d[:])
                bet_row = consts.tile([1, D], f32, tag="bet_row", name="bet_row")
                nc.sync.dma_start(out=bet_row[:], in_=beta_d[:])
                gbc_sb = consts.tile([128, D], f32, tag="gbc", name="gbc_sb")
                bbc_sb = consts.tile([128, D], f32, tag="bbc", name="bbc_sb")
                with tc.tile_pool(name="ps_setup", bufs=1, space="PSUM") as ps_setup:
                    for row, dst, nm in ((gam_row, gbc_sb, "g"), (bet_row, bbc_sb, "b")):
                        bc_ps = ps_setup.tile([128, D], f32, tag="lnbc", name=f"lnbc_{nm}")
                        for e in range(2):
                            nc.tensor.matmul(
                                bc_ps[:, e * 512 : (e + 1) * 512],
                                ones_sb[0:1, :],
                                row[:, e * 512 : (e + 1) * 512],
                                start=True,
                                stop=True,
                            )
                        nc.vector.tensor_copy(out=dst[:], in_=bc_ps[:])

            # ---- projections ----------------------------------------------
            # QhT/KhT: [256(d), 2048(i)] as two [128, 2048] tiles (d on partitions,
            # head pair per tile). Vh: 16 token tiles [128(j), 256(d)].
            qht_sb = [
                persist.tile([128, S], f32, tag=f"qht{t}", name=f"qht{t}") for t in range(2)
            ]
            kht_sb = [
                persist.tile([128, S], f32, tag=f"kht{t}", name=f"kht{t}") for t in range(2)
            ]
            vh_sb = [
                persist.tile([128, DPC], f32, tag=f"vh{j}", name=f"vh{j}") for j in range(16)
            ]

            with tc.tile_pool(name="ps_proj", bufs=8, space="PSUM") as ps_proj:
                for which, wsb, dsrc, dst in (
                    ("k", wk_sb, xt_k, kht_sb),
                    ("q", wq_sb, xt_q, qht_sb),
                ):
                    ps = [
                        ps_proj.tile([128, 512], f32, tag="proj", name=f"ps_{which}{i}")
                        for i in range(8)
                    ]
                    for ci in range(NC_TILES):
                        xt_t = stream.tile([128, S], f32, tag="xt", name=f"xt_{which}{ci}")
                        nc.sync.dma_start(out=xt_t[:], in_=dsrc[ci * 128 : (ci + 1) * 128, :])
                        for t in range(2):
                            for s4 in range(4):
                                nc.tensor.matmul(
                                    ps[t * 4 + s4][:],
                                    wsb[ci][:, t * 128 : (t + 1) * 128],
                                    xt_t[:, s4 * 512 : (s4 + 1) * 512],
                                    start=(ci == 0),
                                    stop=(ci == NC_TILES - 1),
                                )
                    for t in range(2):
                        for s4 in range(4):
                            nc.vector.tensor_copy(
                                out=dst[t][:, s4 * 512 : (s4 + 1) * 512],
                                in_=ps[t * 4 + s4][:],
                            )

                # V: one accumulation group per PSUM bank (two interleaved
                # groups in one bank break: a later group's start= wipes the
                # earlier group's partials on the shared partitions). Two
                # rounds of 8 token-tiles; xt_v is streamed twice.
                for half in range(2):
                    vps = [
                        ps_proj.tile([128, 512], f32, tag="proj", name=f"ps_v{half}_{i}")
                        for i in range(8)
                    ]
                    for ci in range(NC_TILES):
                        xt_t = stream.tile([128, S], f32, tag="xt", name=f"xt_v{half}_{ci}")
                        nc.sync.dma_start(out=xt_t[:], in_=xt_v[ci * 128 : (ci + 1) * 128, :])
                        for kk in range(8):
                            it = half * 8 + kk
                            nc.tensor.matmul(
                                vps[kk][:, 0:DPC],
                                xt_t[:, it * 128 : (it + 1) * 128],
                                wv_sb[ci][:],
                                start=(ci == 0),
                                stop=(ci == NC_TILES - 1),
                            )
                    for kk in range(8):
                        nc.vector.tensor_copy(
                            out=vh_sb[half * 8 + kk][:], in_=vps[kk][:, 0:DPC]
                        )

            if debug:
                for t in range(2):
                    nc.sync.dma_start(out=dbg["qht"][t], in_=qht_sb[t][:])
                    nc.sync.dma_start(out=dbg["kht"][t], in_=kht_sb[t][:])
                for j in range(16):
                    nc.sync.dma_start(out=dbg["vh"][j], in_=vh_sb[j][:])

            # ---- attention -------------------------------------------------
            # og_in[dest_rank, d_local, i_slab]: our scaled OT block for each
            # destination (duplicated across the two batch groups).
            og_in = dram.tile([NCORES, DPC, SL], f32, tag="og_in", name="og_in")
            og_out = dram.tile([NCORES, DPC, SL], f32, tag="og_out", name="og_out")

            with tc.tile_pool(name="ps_attn", bufs=1, space="PSUM") as ps_attn:
                for pair in range(2):  # head pair (2 heads each)
                    for s in range(4):  # 512-query slab
                        ot_ps = ps_attn.tile(
                            [128, 512], f32, tag="ot", bufs=2, name=f"ot{pair}{s}"
                        )
                        # both heads' key-sums at partition 0 (bank per head):
                        # custom-DVE reciprocal misbehaves on partition-base!=0
                        sums_ps = ps_attn.tile(
                            [1, 1024], f32, tag="sums", bufs=1, name=f"sums{pair}{s}"
                        )
                        ex_tiles = {}
                        for jj in range(17):  # 16 key tiles, software-skewed by 1
                            if jj < 16:
                                sc = ps_attn.tile(
                                    [128, 1024], f32, tag="sc", bufs=2, name=f"sc{pair}{s}{jj}"
                                )
                                for hi in range(2):
                                    nc.tensor.matmul(
                                        sc[:, hi * 512 : (hi + 1) * 512],
                                        kht_sb[pair][
                                            hi * 64 : (hi + 1) * 64, jj * 128 : (jj + 1) * 128
                                        ],
                                        qht_sb[pair][
                                            hi * 64 : (hi + 1) * 64, s * 512 : (s + 1) * 512
                                        ],
                                        start=True,
                                        stop=True,
                                    )
                                ex = work.tile(
                                    [128, 1024], f32, tag="ex", bufs=3, name=f"ex{pair}{s}{jj}"
                                )
                                nc.scalar.activation(
                                    out=ex[:], in_=sc[:], func=AF.Exp, scale=1.0 / (DK**0.5)
                                )
                                if debug and pair == 0 and s == 0 and jj == 0:
                                    nc.sync.dma_start(out=dbg["ex"][:], in_=ex[:])
                                ex_tiles[jj] = ex
                            if jj >= 1:
                                j0 = jj - 1
                                ex = ex_tiles.pop(j0)
                                for hi in range(2):
                                    nc.tensor.matmul(
                                        ot_ps[hi * 64 : (hi + 1) * 64, :],
                                        vh_sb[j0][:, pair * 128 + hi * 64 : pair * 128 + (hi + 1) * 64],
                                        ex[:, hi * 512 : (hi + 1) * 512],
                                        start=(j0 == 0),
                                        stop=(j0 == 15),
                                    )
                                for hi in range(2):
                                    nc.tensor.matmul(
                                        sums_ps[0:1, hi * 512 : (hi + 1) * 512],
                                        ones_sb[:, 0:1],
                                        ex[:, hi * 512 : (hi + 1) * 512],
                                        start=(j0 == 0),
                                        stop=(j0 == 15),
                                    )

                        # softmax normalization: r = 1/sums, broadcast over
                        # partitions via a K=1 ones-matmul, multiply into OT.
                        r_sb = work.tile([1, 1024], f32, tag="r", bufs=2, name=f"r{pair}{s}")
                        r_scr = work.tile([1, 1024], f32, tag="rscr", bufs=2, name=f"rs{pair}{s}")
                        nc.vector.reciprocal_approx_accurate(
                            out=r_sb[:], in_=sums_ps[:], scratch=r_scr[:]
                        )
                        bc_ps = ps_attn.tile(
                            [128, 1024], f32, tag="sc", bufs=2, name=f"bc{pair}{s}"
                        )
                        for hi in range(2):
                            nc.tensor.matmul(
                                bc_ps[hi * 64 : (hi + 1) * 64, 0:512],
                                ones_sb[0:1, 0:64],
                                r_sb[0:1, hi * 512 : (hi + 1) * 512],
                                start=True,
                                stop=True,
                            )
                        bc_sb = work.tile([128, 512], f32, tag="bc", bufs=2, name=f"bc_sb{pair}{s}")
                        nc.vector.tensor_copy(out=bc_sb[:], in_=bc_ps[:, 0:512])
                        osc = work.tile([128, 512], f32, tag="osc", bufs=2, name=f"osc{pair}{s}")
                        nc.vector.tensor_mul(out=osc[:], in0=ot_ps[:], in1=bc_sb[:])
                        if debug and pair == 0 and s == 0:
                            nc.sync.dma_start(out=dbg["r"][:], in_=r_sb[:])
                            nc.sync.dma_start(out=dbg["bc"][:], in_=bc_sb[:])
                            dot = work.tile([128, 512], f32, tag="dbgot", name="dbg_ot_sb")
                            nc.vector.tensor_copy(out=dot[:], in_=ot_ps[:])
                            nc.sync.dma_start(out=dbg["ot"][:], in_=dot[:])
                            dsm = work.tile([1, 1024], f32, tag="dbgsm", name="dbg_sm_sb")
                            nc.vector.tensor_copy(out=dsm[:], in_=sums_ps[:])
                            nc.sync.dma_start(out=dbg["sums"][:], in_=dsm[:])
                            nc.sync.dma_start(out=dbg["osc"][:], in_=osc[:])
                        for grp in range(2):
                            nc.sync.dma_start(
                                out=og_in[grp * 4 + s, pair * 128 : (pair + 1) * 128, :],
                                in_=osc[:],
                            )

            # ---- exchange --------------------------------------------------
            nc.gpsimd.collective_compute(
                "AllToAll",
                OP.bypass,
                replica_groups=[list(range(NCORES))],
                ins=[og_in.opt()],
                outs=[og_out.opt()],
            )
            if debug:
                nc.sync.dma_start(out=dbg["ogout"][:], in_=og_out[:])

            # ---- fc + residual + LayerNorm ---------------------------------
            with tc.tile_pool(name="ps_fc", bufs=1, space="PSUM") as ps_fc:
                fc_ps = [
                    ps_fc.tile([128, 512], f32, tag="fc", bufs=8, name=f"fc{i}")
                    for i in range(8)
                ]
                for t in range(16):  # extended contraction: rank r block, d tile
                    otx = stream.tile([128, 512], f32, tag="otx", bufs=3, name=f"otx{t}")
                    nc.sync.dma_start(
                        out=otx[:], in_=og_out[t // 2, (t % 2) * 128 : (t % 2) * 128 + 128, :]
                    )
                    wfx = stream.tile([128, D], f32, tag="wfx", bufs=3, name=f"wfx{t}")
                    nc.sync.dma_start(out=wfx[:], in_=wfcx_d[t * 128 : (t + 1) * 128, :])
                    for it in range(4):
                        for e in range(2):
                            nc.tensor.matmul(
                                fc_ps[it * 2 + e][:],
                                otx[:, it * 128 : (it + 1) * 128],
                                wfx[:, e * 512 : (e + 1) * 512],
                                start=(t == 0),
                                stop=(t == 15),
                            )

                for it in range(4):
                    res_t = stream.tile([128, D], f32, tag="res", bufs=2, name=f"res{it}")
                    nc.sync.dma_start(out=res_t[:], in_=resid_d[it * 128 : (it + 1) * 128, :])
                    y = work.tile([128, D], f32, tag="y", bufs=2, name=f"y{it}")
                    for e in range(2):
                        nc.vector.tensor_add(
                            out=y[:, e * 512 : (e + 1) * 512],
                            in0=fc_ps[it * 2 + e][:],
                            in1=res_t[:, e * 512 : (e + 1) * 512],
                        )
                    st = work.tile([128, 2, 6], f32, tag="st", bufs=2, name=f"st{it}")
                    for e in range(2):
                        nc.vector.bn_stats(out=st[:, e, :], in_=y[:, e * 512 : (e + 1) * 512])
                    mv = work.tile([128, 2], f32, tag="mv", bufs=2, name=f"mv{it}")
                    nc.vector.bn_aggr(out=mv[:], in_=st[:])
                    # rstd = 1/sqrt(var+eps); nmr = -mu*rstd
                    sd = work.tile([128, 1], f32, tag="sd", bufs=2, name=f"sd{it}")
                    nc.scalar.activation(
                        out=sd[:], in_=mv[:, 1:2], func=AF.Sqrt, bias=eps_sb[:], scale=1.0
                    )
                    rstd = work.tile([128, 1], f32, tag="rstd", bufs=2, name=f"rstd{it}")
                    nc.vector.reciprocal(out=rstd[:], in_=sd[:])
                    nmr = work.tile([128, 1], f32, tag="nmr", bufs=2, name=f"nmr{it}")
                    nc.vector.tensor_scalar(
                        out=nmr[:],
                        in0=mv[:, 0:1],
                        scalar1=rstd[:],
                        scalar2=-1.0,
                        op0=OP.mult,
                        op1=OP.mult,
                    )
                    z = work.tile([128, D], f32, tag="z", bufs=2, name=f"z{it}")
                    nc.vector.tensor_scalar(
                        out=z[:],
                        in0=y[:],
                        scalar1=rstd[:],
                        scalar2=nmr[:],
                        op0=OP.mult,
                        op1=OP.add,
                    )
                    if not trivial_ln:
                        z2 = work.tile([128, D], f32, tag="z2", bufs=2, name=f"z2{it}")
                        nc.vector.tensor_mul(out=z2[:], in0=z[:], in1=gbc_sb[:])
                        z3 = work.tile([128, D], f32, tag="z3", bufs=2, name=f"z3{it}")
                        nc.vector.tensor_add(out=z3[:], in0=z2[:], in1=bbc_sb[:])
                        z = z3
                    nc.sync.dma_start(out=out_d[it * 128 : (it + 1) * 128, :], in_=z[:])

    nc.compile()
    return nc


def _get_nc(trivial_ln: bool, debug: bool = False):
    key = ("nc", trivial_ln, debug)
    if key not in _CACHE:
        _CACHE[key] = _build(trivial_ln, debug)
    return _CACHE[key]


def _shard(inputs):
    q = np.ascontiguousarray(np.asarray(inputs["q"], dtype=np.float32))
    k = np.ascontiguousarray(np.asarray(inputs["k"], dtype=np.float32))
    v = np.ascontiguousarray(np.asarray(inputs["v"], dtype=np.float32))
    w_q = np.asarray(inputs["w_q"], dtype=np.float32)
    w_k = np.asarray(inputs["w_k"], dtype=np.float32)
    w_v = np.asarray(inputs["w_v"], dtype=np.float32)
    w_fc = np.asarray(inputs["w_fc"], dtype=np.float32)
    gamma = np.asarray(inputs["ln_gamma"], dtype=np.float32).reshape(1, D)
    beta = np.asarray(inputs["ln_beta"], dtype=np.float32).reshape(1, D)

    xt = {}
    for gi in range(2):
        xt[gi] = (
            np.ascontiguousarray(q[gi].T),
            np.ascontiguousarray(k[gi].T),
            np.ascontiguousarray(v[gi].T),
        )
    wq_s = [np.ascontiguousarray(w_q[:, p * DPC : (p + 1) * DPC]) for p in range(PG)]
    wk_s = [np.ascontiguousarray(w_k[:, p * DPC : (p + 1) * DPC]) for p in range(PG)]
    wv_s = [np.ascontiguousarray(w_v[:, p * DPC : (p + 1) * DPC]) for p in range(PG)]
    # extended fc weights: row block r (256 rows) = w_fc rows for rank r's heads,
    # zeroed when rank r belongs to the other batch group
    wfcx = []
    for gi in range(2):
        w = np.zeros((2 * D, D), dtype=np.float32)
        for r in range(NCORES):
            if r // PG == gi:
                hp = r % PG
                w[r * DPC : (r + 1) * DPC, :] = w_fc[hp * DPC : (hp + 1) * DPC, :]
        wfcx.append(w)

    in_maps = []
    for c in range(NCORES):
        gi, p = divmod(c, PG)
        in_maps.append(
            {
                "xt_q": xt[gi][0],
                "xt_k": xt[gi][1],
                "xt_v": xt[gi][2],
                "wq": wq_s[p],
                "wk": wk_s[p],
                "wv": wv_s[p],
                "wfcx": wfcx[gi],
                "resid": np.ascontiguousarray(q[gi, p * SL : (p + 1) * SL, :]),
                "gamma": gamma,
                "beta": beta,
            }
        )
    trivial_ln = bool(np.all(gamma == 1.0) and np.all(beta == 0.0))
    return in_maps, trivial_ln


def _run(inputs, trace=False, debug=False):
    from concourse.bass_utils import run_bass_kernel_spmd

    in_maps, trivial_ln = _shard(inputs)
    nc = _get_nc(trivial_ln, debug)
    res = run_bass_kernel_spmd(
        nc, in_maps, core_ids=list(range(NCORES)), trace=trace
    )
    out = np.empty((B, S, D), dtype=np.float32)
    for c in range(NCORES):
        gi, p = divmod(c, PG)
        out[gi, p * SL : (p + 1) * SL, :] = res.results[c]["out"]
    return out, res


def kernel(**inputs) -> np.ndarray:
    out, _ = _run(inputs)
    return out


def _timed_exec(inputs, iters=5):
    """Execute on 8 cores with device-resident inputs; return (out, [dt_ns])."""
    import time

    import jax
    import jax.numpy as jnp  # noqa: F401
    from jax.sharding import Mesh, PartitionSpec
    from jax.experimental.shard_map import shard_map

    import concourse.mybir as mybir
    from concourse import bass2jax

    in_maps, trivial_ln = _shard(inputs)
    nc = _get_nc(trivial_ln)
    bass2jax.install_neuronx_cc_hook()

    n_cores = NCORES
    partition_name = nc.partition_id_tensor.name if nc.partition_id_tensor else None
    in_names, out_names, out_avals, zero_outs = [], [], [], []
    for alloc in nc.m.functions[0].allocations:
        if not isinstance(alloc, mybir.MemoryLocationSet):
            continue
        name = alloc.memorylocations[0].name
        if alloc.kind == "ExternalInput":
            if name != partition_name:
                in_names.append(name)
        elif alloc.kind == "ExternalOutput":
            shape = tuple(alloc.tensor_shape)
            dtype = mybir.dt.np(alloc.dtype)
            out_names.append(name)
            out_avals.append(jax.core.ShapedArray(shape, dtype))
            zero_outs.append(np.zeros(shape, dtype))
    n_params = len(in_names)
    n_outs = len(out_avals)
    all_names = in_names + out_names
    if partition_name is not None:
        all_names = all_names + [partition_name]
    donate = tuple(range(n_params, n_params + n_outs))

    def _body(*args):
        operands = list(args)
        if partition_name is not None:
            operands.append(bass2jax.partition_id_tensor())
        outs = bass2jax._bass_exec_p.bind(
            *operands,
            out_avals=tuple(out_avals),
            in_names=tuple(all_names),
            out_names=tuple(out_names),
            lowering_input_output_aliases=(),
            sim_require_finite=True,
            sim_require_nnan=True,
            nc=nc,
        )
        return tuple(outs)

    devices = jax.devices()[:n_cores]
    mesh = Mesh(np.asarray(devices), ("core",))
    in_specs = (PartitionSpec("core"),) * (n_params + n_outs)
    out_specs = (PartitionSpec("core"),) * n_outs
    sharded = jax.jit(
        shard_map(_body, mesh=mesh, in_specs=in_specs, out_specs=out_specs, check_rep=False),
        donate_argnums=donate,
        keep_unused=True,
    )
    from jax.sharding import NamedSharding

    shd = NamedSharding(mesh, PartitionSpec("core"))
    concat_in = [
        jax.device_put(
            np.concatenate([np.asarray(in_maps[c][n]) for c in range(n_cores)], axis=0), shd
        )
        for n in in_names
    ]
    times = []
    out_arrs = None
    for _ in range(iters):
        zeros_dev = [
            jax.device_put(np.zeros((n_cores * z.shape[0], *z.shape[1:]), z.dtype), shd)
            for z in zero_outs
        ]
        jax.block_until_ready(zeros_dev)
        t0 = time.perf_counter()
        out_arrs = sharded(*concat_in, *zeros_dev)
        jax.block_until_ready(out_arrs)
        times.append((time.perf_counter() - t0) * 1e9)

    # pipelined: many dispatches in flight so the axon RPC overhead overlaps;
    # marginal per-call time approximates device execution time
    npipe = 16
    zsets = [
        [
            jax.device_put(np.zeros((n_cores * z.shape[0], *z.shape[1:]), z.dtype), shd)
            for z in zero_outs
        ]
        for _ in range(npipe)
    ]
    jax.block_until_ready(zsets)
    outs = [sharded(*concat_in, *zsets[0])]
    jax.block_until_ready(outs[0])  # warm
    t0 = time.perf_counter()
    for i in range(1, npipe):
        outs.append(sharded(*concat_in, *zsets[i]))
    jax.block_until_ready(outs)
    tp = (time.perf_counter() - t0) * 1e9 / (npipe - 1)
    times.append(tp)
    out = np.empty((B, S, D), dtype=np.float32)
    full = np.asarray(out_arrs[out_names.index("out")]).reshape(n_cores, SL, D)
    for c in range(n_cores):
        gi, p = divmod(c, PG)
        out[gi, p * SL : (p + 1) * SL, :] = full[c]
    return out, times


def _dispatch_floor(iters=5):
    """Measure the axon dispatch floor with a trivial jitted op on all 8 devices."""
    import time

    import jax
    import jax.numpy as jnp
    from jax.sharding import Mesh, PartitionSpec, NamedSharding

    devices = jax.devices()[:NCORES]
    mesh = Mesh(np.asarray(devices), ("core",))
    shd = NamedSharding(mesh, PartitionSpec("core"))
    x = jax.device_put(np.ones((NCORES, 8), np.float32), shd)
    f = jax.jit(lambda a: a + 1.0)
    jax.block_until_ready(f(x))
    times = []
    for _ in range(iters):
        t0 = time.perf_counter()
        jax.block_until_ready(f(x))
        times.append((time.perf_counter() - t0) * 1e9)
    return times


# revision 29
# speedup vs baseline: 2.4067x; 2.4067x over previous
"""Trainium2 Bass kernel for nn_MultiHeadAttention (B=2, S=2048, D=1024, H=16, dk=dv=64).

Sharding: 8 cores = 2 batch groups x 4 cores. Core c (g=c//4, p=c%4) computes
attention for 4 heads (heads p*4..p*4+3) of batch g over all 2048 tokens, then
an 8-rank AllToAll redistributes the attention output so core c holds all 16
heads for its 512-token slice; fc + residual + LayerNorm run token-parallel.

Layout choices (all matmul contractions live on the partition axis):
  - host supplies q/k/v transposed (XT[c, i]) so projections need no on-device
    transpose: QhT/KhT = W.T-free (lhsT=W tiles), Vh token-major (lhsT=XT tiles)
  - scores are computed transposed, ST[j, i] (keys on partitions), two heads at
    a time via row-tiled K=64 matmuls; exp runs on ACT with scale=1/sqrt(dk)
  - attention output accumulates as OT[d, i] via col-tiled M=64 matmuls
    (lhsT = Vh), softmax denominators ride along as col-tiled M=1 ones-matmuls
  - normalization: r=1/sums broadcast across partitions with a K=1 ones-matmul,
    then one DVE multiply
  - fc contracts over an extended 2048-row axis (both groups' AllToAll blocks);
    the host zeroes the other group's rows of w_fc so no per-rank addressing is
    needed (the program is identical on all 8 cores).
"""

import os
import sys

import numpy as np

if "/opt/trn_rl_repo" not in sys.path:
    sys.path.insert(0, "/opt/trn_rl_repo")

B, S, D = 2, 2048, 1024
H, DK, DV = 16, 64, 64
LN_EPS = 1e-5

NCORES = 8
PG = 4          # cores per batch group
HPC = 4         # heads per core
DPC = HPC * DV  # 256 local output dims per core
SL = S // PG    # 512 tokens per core for fc/LN
NC_TILES = D // 128  # 8 contraction tiles for projections

_CACHE = {}


def _build(trivial_ln: bool, debug: bool = False):
    import concourse.bass as bass  # noqa: F401
    import concourse.mybir as mybir
    import concourse.tile as tile
    from concourse import bacc

    f32 = mybir.dt.float32
    bf16 = mybir.dt.bfloat16
    AF = mybir.ActivationFunctionType
    OP = mybir.AluOpType

    nc = bacc.Bacc()

    xt_q = nc.dram_tensor("xt_q", [D, S], bf16, kind="ExternalInput")
    xt_k = nc.dram_tensor("xt_k", [D, S], bf16, kind="ExternalInput")
    xt_v = nc.dram_tensor("xt_v", [D, S], bf16, kind="ExternalInput")
    wq_d = nc.dram_tensor("wq", [D, DPC], bf16, kind="ExternalInput")
    wk_d = nc.dram_tensor("wk", [D, DPC], bf16, kind="ExternalInput")
    wv_d = nc.dram_tensor("wv", [D, DPC], bf16, kind="ExternalInput")
    wfcx_d = nc.dram_tensor("wfcx", [2 * D, D], bf16, kind="ExternalInput")
    resid_d = nc.dram_tensor("resid", [SL, D], f32, kind="ExternalInput")
    gamma_d = nc.dram_tensor("gamma", [1, D], f32, kind="ExternalInput")
    beta_d = nc.dram_tensor("beta", [1, D], f32, kind="ExternalInput")
    out_d = nc.dram_tensor("out", [SL, D], f32, kind="ExternalOutput")
    dbg = {}
    if debug:
        bf16_ = mybir.dt.bfloat16
        dbg["qht"] = nc.dram_tensor("dbg_qht", [2, 128, S], bf16_, kind="ExternalOutput")
        dbg["kht"] = nc.dram_tensor("dbg_kht", [2, 128, S], bf16_, kind="ExternalOutput")
        dbg["vh"] = nc.dram_tensor("dbg_vh", [16, 128, DPC], bf16_, kind="ExternalOutput")
        dbg["ex"] = nc.dram_tensor("dbg_ex", [128, 1024], bf16_, kind="ExternalOutput")
        dbg["ot"] = nc.dram_tensor("dbg_ot", [128, 512], f32, kind="ExternalOutput")
        dbg["sums"] = nc.dram_tensor("dbg_sums", [1, 1024], f32, kind="ExternalOutput")
        dbg["osc"] = nc.dram_tensor("dbg_osc", [128, 512], bf16_, kind="ExternalOutput")
        dbg["ogout"] = nc.dram_tensor(
            "dbg_ogout", [NCORES, DPC, SL], bf16_, kind="ExternalOutput"
        )
        dbg["r"] = nc.dram_tensor("dbg_r", [1, 1024], f32, kind="ExternalOutput")
        dbg["bc"] = nc.dram_tensor("dbg_bc", [128, 512], f32, kind="ExternalOutput")

    with tile.TileContext(nc) as tc:
        with (
            tc.tile_pool(name="consts", bufs=1) as consts,
            tc.tile_pool(name="persist", bufs=1) as persist,
            tc.tile_pool(name="stream", bufs=3) as stream,
            tc.tile_pool(name="work", bufs=3) as work,
            tc.tile_pool(name="dram", bufs=1, space="DRAM") as dram,
        ):
            ones_sb = consts.tile([128, 128], f32, tag="ones", name="ones_sb")
            nc.vector.memset(ones_sb[:], 1.0)
            ones_bf = consts.tile([128, 128], bf16, tag="onesbf", name="ones_bf")
            nc.vector.memset(ones_bf[:], 1.0)
            eps_sb = consts.tile([128, 1], f32, tag="eps", name="eps_sb")
            nc.vector.memset(eps_sb[:], LN_EPS)

            # ---- weights for the projections
            wq_sb, wk_sb, wv_sb = [], [], []
            for ci in range(NC_TILES):
                for lst, dsrc, nm in (
                    (wq_sb, wq_d, "wq"),
                    (wk_sb, wk_d, "wk"),
                    (wv_sb, wv_d, "wv"),
                ):
                    t = persist.tile([128, DPC], bf16, tag=f"{nm}{ci}", name=f"{nm}{ci}_sb")
                    nc.sync.dma_start(out=t[:], in_=dsrc[ci * 128 : (ci + 1) * 128, :])
                    lst.append(t)

            # ---- gamma/beta broadcast tiles (general LN path only)
            gbc_sb = bbc_sb = None
            if not trivial_ln:
                gam_row = consts.tile([1, D], f32, tag="gam_row", name="gam_row")
                nc.sync.dma_start(out=gam_row[:], in_=gamma_d[:])
                bet_row = consts.tile([1, D], f32, tag="bet_row", name="bet_row")
                nc.sync.dma_start(out=bet_row[:], in_=beta_d[:])
                gbc_sb = consts.tile([128, D], f32, tag="gbc", name="gbc_sb")
                bbc_sb = consts.tile([128, D], f32, tag="bbc", name="bbc_sb")
                with tc.tile_pool(name="ps_setup", bufs=1, space="PSUM") as ps_setup:
                    for row, dst, nm in ((gam_row, gbc_sb, "g"), (bet_row, bbc_sb, "b")):
                        bc_ps = ps_setup.tile([128, D], f32, tag="lnbc", name=f"lnbc_{nm}")
                        for e in range(2):
                            nc.tensor.matmul(
                                bc_ps[:, e * 512 : (e + 1) * 512],
                                ones_sb[0:1, :],
                                row[:, e * 512 : (e + 1) * 512],
                                start=True,
                                stop=True,
                            )
                        nc.vector.tensor_copy(out=dst[:], in_=bc_ps[:])

            # ---- projections ----------------------------------------------
            # QhT/KhT: [256(d), 2048(i)] as two [128, 2048] tiles (d on partitions,
            # head pair per tile). Vh: 16 token tiles [128(j), 256(d)].
            qht_sb = [
                persist.tile([128, S], bf16, tag=f"qht{t}", name=f"qht{t}") for t in range(2)
            ]
            kht_sb = [
                persist.tile([128, S], bf16, tag=f"kht{t}", name=f"kht{t}") for t in range(2)
            ]
            vh_sb = [
                persist.tile([128, DPC], bf16, tag=f"vh{j}", name=f"vh{j}") for j in range(16)
            ]

            with tc.tile_pool(name="ps_proj", bufs=8, space="PSUM") as ps_proj:
                for which, wsb, dsrc, dst in (
                    ("k", wk_sb, xt_k, kht_sb),
                    ("q", wq_sb, xt_q, qht_sb),
                ):
                    ps = [
                        ps_proj.tile([128, 512], f32, tag="proj", name=f"ps_{which}{i}")
                        for i in range(8)
                    ]
                    for ci in range(NC_TILES):
                        xt_t = stream.tile([128, S], bf16, tag="xt", name=f"xt_{which}{ci}")
                        nc.sync.dma_start(out=xt_t[:], in_=dsrc[ci * 128 : (ci + 1) * 128, :])
                        for t in range(2):
                            for s4 in range(4):
                                nc.tensor.matmul(
                                    ps[t * 4 + s4][:],
                                    wsb[ci][:, t * 128 : (t + 1) * 128],
                                    xt_t[:, s4 * 512 : (s4 + 1) * 512],
                                    start=(ci == 0),
                                    stop=(ci == NC_TILES - 1),
                                )
                    for t in range(2):
                        for s4 in range(4):
                            nc.vector.tensor_copy(
                                out=dst[t][:, s4 * 512 : (s4 + 1) * 512],
                                in_=ps[t * 4 + s4][:],
                            )

                # V: one accumulation group per PSUM bank (two interleaved
                # groups in one bank break: a later group's start= wipes the
                # earlier group's partials on the shared partitions). Two
                # rounds of 8 token-tiles; xt_v is streamed twice.
                for half in range(2):
                    vps = [
                        ps_proj.tile([128, 512], f32, tag="proj", name=f"ps_v{half}_{i}")
                        for i in range(8)
                    ]
                    for ci in range(NC_TILES):
                        xt_t = stream.tile([128, S], bf16, tag="xt", name=f"xt_v{half}_{ci}")
                        nc.sync.dma_start(out=xt_t[:], in_=xt_v[ci * 128 : (ci + 1) * 128, :])
                        for kk in range(8):
                            it = half * 8 + kk
                            nc.tensor.matmul(
                                vps[kk][:, 0:DPC],
                                xt_t[:, it * 128 : (it + 1) * 128],
                                wv_sb[ci][:],
                                start=(ci == 0),
                                stop=(ci == NC_TILES - 1),
                            )
                    for kk in range(8):
                        nc.vector.tensor_copy(
                            out=vh_sb[half * 8 + kk][:], in_=vps[kk][:, 0:DPC]
                        )

            if debug:
                for t in range(2):
                    nc.sync.dma_start(out=dbg["qht"][t], in_=qht_sb[t][:])
                    nc.sync.dma_start(out=dbg["kht"][t], in_=kht_sb[t][:])
                for j in range(16):
                    nc.sync.dma_start(out=dbg["vh"][j], in_=vh_sb[j][:])

            # ---- attention -------------------------------------------------
            # og_in[dest_rank, d_local, i_slab]: our scaled OT block for each
            # destination (duplicated across the two batch groups).
            og_in = dram.tile([NCORES, DPC, SL], bf16, tag="og_in", name="og_in")
            og_out = dram.tile([NCORES, DPC, SL], bf16, tag="og_out", name="og_out")

            with tc.tile_pool(name="ps_attn", bufs=1, space="PSUM") as ps_attn:
                for pair in range(2):  # head pair (2 heads each)
                    for s in range(4):  # 512-query slab
                        ot_ps = ps_attn.tile(
                            [128, 512], f32, tag="ot", bufs=2, name=f"ot{pair}{s}"
                        )
                        # both heads' key-sums at partition 0 (bank per head):
                        # custom-DVE reciprocal misbehaves on partition-base!=0
                        sums_ps = ps_attn.tile(
                            [1, 1024], f32, tag="sums", bufs=1, name=f"sums{pair}{s}"
                        )
                        ex_tiles = {}
                        for jj in range(17):  # 16 key tiles, software-skewed by 1
                            if jj < 16:
                                sc = ps_attn.tile(
                                    [128, 1024], f32, tag="sc", bufs=2, name=f"sc{pair}{s}{jj}"
                                )
                                for hi in range(2):
                                    nc.tensor.matmul(
                                        sc[:, hi * 512 : (hi + 1) * 512],
                                        kht_sb[pair][
                                            hi * 64 : (hi + 1) * 64, jj * 128 : (jj + 1) * 128
                                        ],
                                        qht_sb[pair][
                                            hi * 64 : (hi + 1) * 64, s * 512 : (s + 1) * 512
                                        ],
                                        start=True,
                                        stop=True,
                                    )
                                ex = work.tile(
                                    [128, 1024], bf16, tag="ex", bufs=3, name=f"ex{pair}{s}{jj}"
                                )
                                nc.scalar.activation(
                                    out=ex[:], in_=sc[:], func=AF.Exp, scale=1.0 / (DK**0.5)
                                )
                                if debug and pair == 0 and s == 0 and jj == 0:
                                    nc.sync.dma_start(out=dbg["ex"][:], in_=ex[:])
                                ex_tiles[jj] = ex
                            if jj >= 1:
                                j0 = jj - 1
                                ex = ex_tiles.pop(j0)
                                for hi in range(2):
                                    nc.tensor.matmul(
                                        ot_ps[hi * 64 : (hi + 1) * 64, :],
                                        vh_sb[j0][:, pair * 128 + hi * 64 : pair * 128 + (hi + 1) * 64],
                                        ex[:, hi * 512 : (hi + 1) * 512],
                                        start=(j0 == 0),
                                        stop=(j0 == 15),
                                    )
                                for hi in range(2):
                                    nc.tensor.matmul(
                                        sums_ps[0:1, hi * 512 : (hi + 1) * 512],
                                        ones_bf[:, 0:1],
                                        ex[:, hi * 512 : (hi + 1) * 512],
                                        start=(j0 == 0),
                                        stop=(j0 == 15),
                                    )

                        # softmax normalization: r = 1/sums, broadcast over
                        # partitions via a K=1 ones-matmul, multiply into OT.
                        r_sb = work.tile([1, 1024], f32, tag="r", bufs=2, name=f"r{pair}{s}")
                        r_scr = work.tile([1, 1024], f32, tag="rscr", bufs=2, name=f"rs{pair}{s}")
                        nc.vector.reciprocal_approx_accurate(
                            out=r_sb[:], in_=sums_ps[:], scratch=r_scr[:]
                        )
                        bc_ps = ps_attn.tile(
                            [128, 1024], f32, tag="sc", bufs=2, name=f"bc{pair}{s}"
                        )
                        for hi in range(2):
                            nc.tensor.matmul(
                                bc_ps[hi * 64 : (hi + 1) * 64, 0:512],
                                ones_sb[0:1, 0:64],
                                r_sb[0:1, hi * 512 : (hi + 1) * 512],
                                start=True,
                                stop=True,
                            )
                        bc_sb = work.tile([128, 512], f32, tag="bc", bufs=2, name=f"bc_sb{pair}{s}")
                        nc.vector.tensor_copy(out=bc_sb[:], in_=bc_ps[:, 0:512])
                        osc = work.tile([128, 512], bf16, tag="osc", bufs=2, name=f"osc{pair}{s}")
                        nc.vector.tensor_mul(out=osc[:], in0=ot_ps[:], in1=bc_sb[:])
                        if debug and pair == 0 and s == 0:
                            nc.sync.dma_start(out=dbg["r"][:], in_=r_sb[:])
                            nc.sync.dma_start(out=dbg["bc"][:], in_=bc_sb[:])
                            dot = work.tile([128, 512], f32, tag="dbgot", name="dbg_ot_sb")
                            nc.vector.tensor_copy(out=dot[:], in_=ot_ps[:])
                            nc.sync.dma_start(out=dbg["ot"][:], in_=dot[:])
                            dsm = work.tile([1, 1024], f32, tag="dbgsm", name="dbg_sm_sb")
                            nc.vector.tensor_copy(out=dsm[:], in_=sums_ps[:])
                            nc.sync.dma_start(out=dbg["sums"][:], in_=dsm[:])
                            nc.sync.dma_start(out=dbg["osc"][:], in_=osc[:])
                        for grp in range(2):
                            nc.sync.dma_start(
                                out=og_in[grp * 4 + s, pair * 128 : (pair + 1) * 128, :],
                                in_=osc[:],
                            )

            # ---- exchange --------------------------------------------------
            nc.gpsimd.collective_compute(
                "AllToAll",
                OP.bypass,
                replica_groups=[list(range(NCORES))],
                ins=[og_in.opt()],
                outs=[og_out.opt()],
            )
            if debug:
                nc.sync.dma_start(out=dbg["ogout"][:], in_=og_out[:])

            # ---- fc + residual + LayerNorm ---------------------------------
            with tc.tile_pool(name="ps_fc", bufs=1, space="PSUM") as ps_fc:
                fc_ps = [
                    ps_fc.tile([128, 512], f32, tag="fc", bufs=8, name=f"fc{i}")
                    for i in range(8)
                ]
                for t in range(16):  # extended contraction: rank r block, d tile
                    otx = stream.tile([128, 512], bf16, tag="otx", bufs=3, name=f"otx{t}")
                    nc.sync.dma_start(
                        out=otx[:], in_=og_out[t // 2, (t % 2) * 128 : (t % 2) * 128 + 128, :]
                    )
                    wfx = stream.tile([128, D], bf16, tag="wfx", bufs=3, name=f"wfx{t}")
                    nc.sync.dma_start(out=wfx[:], in_=wfcx_d[t * 128 : (t + 1) * 128, :])
                    for it in range(4):
                        for e in range(2):
                            nc.tensor.matmul(
                                fc_ps[it * 2 + e][:],
                                otx[:, it * 128 : (it + 1) * 128],
                                wfx[:, e * 512 : (e + 1) * 512],
                                start=(t == 0),
                                stop=(t == 15),
                            )

                for it in range(4):
                    res_t = stream.tile([128, D], f32, tag="res", bufs=2, name=f"res{it}")
                    nc.sync.dma_start(out=res_t[:], in_=resid_d[it * 128 : (it + 1) * 128, :])
                    y = work.tile([128, D], f32, tag="y", bufs=2, name=f"y{it}")
                    for e in range(2):
                        nc.vector.tensor_add(
                            out=y[:, e * 512 : (e + 1) * 512],
                            in0=fc_ps[it * 2 + e][:],
                            in1=res_t[:, e * 512 : (e + 1) * 512],
                        )
                    st = work.tile([128, 2, 6], f32, tag="st", bufs=2, name=f"st{it}")
                    for e in range(2):
                        nc.vector.bn_stats(out=st[:, e, :], in_=y[:, e * 512 : (e + 1) * 512])
                    mv = work.tile([128, 2], f32, tag="mv", bufs=2, name=f"mv{it}")
                    nc.vector.bn_aggr(out=mv[:], in_=st[:])
                    # rstd = 1/sqrt(var+eps); nmr = -mu*rstd
                    sd = work.tile([128, 1], f32, tag="sd", bufs=2, name=f"sd{it}")
                    nc.scalar.activation(
                        out=sd[:], in_=mv[:, 1:2], func=AF.Sqrt, bias=eps_sb[:], scale=1.0
                    )
                    rstd = work.tile([128, 1], f32, tag="rstd", bufs=2, name=f"rstd{it}")
                    nc.vector.reciprocal(out=rstd[:], in_=sd[:])
                    nmr = work.tile([128, 1], f32, tag="nmr", bufs=2, name=f"nmr{it}")
                    nc.vector.tensor_scalar(
                        out=nmr[:],
                        in0=mv[:, 0:1],
                        scalar1=rstd[:],
                        scalar2=-1.0,
                        op0=OP.mult,
                        op1=OP.mult,
                    )
                    z = work.tile([128, D], f32, tag="z", bufs=2, name=f"z{it}")
                    nc.vector.tensor_scalar(
                        out=z[:],
                        in0=y[:],
                        scalar1=rstd[:],
                        scalar2=nmr[:],
                        op0=OP.mult,
                        op1=OP.add,
                    )
                    if not trivial_ln:
                        z2 = work.tile([128, D], f32, tag="z2", bufs=2, name=f"z2{it}")
                        nc.vector.tensor_mul(out=z2[:], in0=z[:], in1=gbc_sb[:])
                        z3 = work.tile([128, D], f32, tag="z3", bufs=2, name=f"z3{it}")
                        nc.vector.tensor_add(out=z3[:], in0=z2[:], in1=bbc_sb[:])
                        z = z3
                    nc.sync.dma_start(out=out_d[it * 128 : (it + 1) * 128, :], in_=z[:])

    nc.compile()
    return nc


def _get_nc(trivial_ln: bool, debug: bool = False):
    key = ("nc", trivial_ln, debug)
    if key not in _CACHE:
        _CACHE[key] = _build(trivial_ln, debug)
    return _CACHE[key]


def _shard(inputs):
    import ml_dtypes

    bf = ml_dtypes.bfloat16
    q = np.ascontiguousarray(np.asarray(inputs["q"], dtype=np.float32))
    k = np.ascontiguousarray(np.asarray(inputs["k"], dtype=np.float32))
    v = np.ascontiguousarray(np.asarray(inputs["v"], dtype=np.float32))
    w_q = np.asarray(inputs["w_q"], dtype=np.float32)
    w_k = np.asarray(inputs["w_k"], dtype=np.float32)
    w_v = np.asarray(inputs["w_v"], dtype=np.float32)
    w_fc = np.asarray(inputs["w_fc"], dtype=np.float32)
    gamma = np.asarray(inputs["ln_gamma"], dtype=np.float32).reshape(1, D)
    beta = np.asarray(inputs["ln_beta"], dtype=np.float32).reshape(1, D)

    xt = {}
    for gi in range(2):
        xt[gi] = (
            np.ascontiguousarray(q[gi].T).astype(bf),
            np.ascontiguousarray(k[gi].T).astype(bf),
            np.ascontiguousarray(v[gi].T).astype(bf),
        )
    wq_s = [np.ascontiguousarray(w_q[:, p * DPC : (p + 1) * DPC]).astype(bf) for p in range(PG)]
    wk_s = [np.ascontiguousarray(w_k[:, p * DPC : (p + 1) * DPC]).astype(bf) for p in range(PG)]
    wv_s = [np.ascontiguousarray(w_v[:, p * DPC : (p + 1) * DPC]).astype(bf) for p in range(PG)]
    # extended fc weights: row block r (256 rows) = w_fc rows for rank r's heads,
    # zeroed when rank r belongs to the other batch group
    wfcx = []
    for gi in range(2):
        w = np.zeros((2 * D, D), dtype=np.float32)
        for r in range(NCORES):
            if r // PG == gi:
                hp = r % PG
                w[r * DPC : (r + 1) * DPC, :] = w_fc[hp * DPC : (hp + 1) * DPC, :]
        wfcx.append(w.astype(bf))

    in_maps = []
    for c in range(NCORES):
        gi, p = divmod(c, PG)
        in_maps.append(
            {
                "xt_q": xt[gi][0],
                "xt_k": xt[gi][1],
                "xt_v": xt[gi][2],
                "wq": wq_s[p],
                "wk": wk_s[p],
                "wv": wv_s[p],
                "wfcx": wfcx[gi],
                "resid": np.ascontiguousarray(q[gi, p * SL : (p + 1) * SL, :]),
                "gamma": gamma,
                "beta": beta,
            }
        )
    trivial_ln = bool(np.all(gamma == 1.0) and np.all(beta == 0.0))
    return in_maps, trivial_ln


def _run(inputs, trace=False, debug=False):
    from concourse.bass_utils import run_bass_kernel_spmd

    in_maps, trivial_ln = _shard(inputs)
    nc = _get_nc(trivial_ln, debug)
    res = run_bass_kernel_spmd(
        nc, in_maps, core_ids=list(range(NCORES)), trace=trace
    )
    out = np.empty((B, S, D), dtype=np.float32)
    for c in range(NCORES):
        gi, p = divmod(c, PG)
        out[gi, p * SL : (p + 1) * SL, :] = res.results[c]["out"]
    return out, res


def kernel(**inputs) -> np.ndarray:
    out, _ = _run(inputs)
    return out


def _timed_exec(inputs, iters=5):
    """Execute on 8 cores with device-resident inputs; return (out, [dt_ns])."""
    import time

    import jax
    import jax.numpy as jnp  # noqa: F401
    from jax.sharding import Mesh, PartitionSpec
    from jax.experimental.shard_map import shard_map

    import concourse.mybir as mybir
    from concourse import bass2jax

    in_maps, trivial_ln = _shard(inputs)
    nc = _get_nc(trivial_ln)
    bass2jax.install_neuronx_cc_hook()

    n_cores = NCORES
    partition_name = nc.partition_id_tensor.name if nc.partition_id_tensor else None
    in_names, out_names, out_avals, zero_outs = [], [], [], []
    for alloc in nc.m.functions[0].allocations:
        if not isinstance(alloc, mybir.MemoryLocationSet):
            continue
        name = alloc.memorylocations[0].name
        if alloc.kind == "ExternalInput":
            if name != partition_name:
                in_names.append(name)
        elif alloc.kind == "ExternalOutput":
            shape = tuple(alloc.tensor_shape)
            dtype = mybir.dt.np(alloc.dtype)
            out_names.append(name)
            out_avals.append(jax.core.ShapedArray(shape, dtype))
            zero_outs.append(np.zeros(shape, dtype))
    n_params = len(in_names)
    n_outs = len(out_avals)
    all_names = in_names + out_names
    if partition_name is not None:
        all_names = all_names + [partition_name]
    donate = tuple(range(n_params, n_params + n_outs))

    def _body(*args):
        operands = list(args)
        if partition_name is not None:
            operands.append(bass2jax.partition_id_tensor())
        outs = bass2jax._bass_exec_p.bind(
            *operands,
            out_avals=tuple(out_avals),
            in_names=tuple(all_names),
            out_names=tuple(out_names),
            lowering_input_output_aliases=(),
            sim_require_finite=True,
            sim_require_nnan=True,
            nc=nc,
        )
        return tuple(outs)

    devices = jax.devices()[:n_cores]
    mesh = Mesh(np.asarray(devices), ("core",))
    in_specs = (PartitionSpec("core"),) * (n_params + n_outs)
    out_specs = (PartitionSpec("core"),) * n_outs
    sharded = jax.jit(
        shard_map(_body, mesh=mesh, in_specs=in_specs, out_specs=out_specs, check_rep=False),
        donate_argnums=donate,
        keep_unused=True,
    )
    from jax.sharding import NamedSharding

    shd = NamedSharding(mesh, PartitionSpec("core"))
    concat_in = [
        jax.device_put(
            np.concatenate([np.asarray(in_maps[c][n]) for c in range(n_cores)], axis=0), shd
        )
        for n in in_names
    ]
    times = []
    out_arrs = None
    for _ in range(iters):
        zeros_dev = [
            jax.device_put(np.zeros((n_cores * z.shape[0], *z.shape[1:]), z.dtype), shd)
            for z in zero_outs
        ]
        jax.block_until_ready(zeros_dev)
        t0 = time.perf_counter()
        out_arrs = sharded(*concat_in, *zeros_dev)
        jax.block_until_ready(out_arrs)
        times.append((time.perf_counter() - t0) * 1e9)

    # pipelined: many dispatches in flight so the axon RPC overhead overlaps;
    # marginal per-call time approximates device execution time
    npipe = 16
    zsets = [
        [
            jax.device_put(np.zeros((n_cores * z.shape[0], *z.shape[1:]), z.dtype), shd)
            for z in zero_outs
        ]
        for _ in range(npipe)
    ]
    jax.block_until_ready(zsets)
    outs = [sharded(*concat_in, *zsets[0])]
    jax.block_until_ready(outs[0])  # warm
    t0 = time.perf_counter()
    for i in range(1, npipe):
        outs.append(sharded(*concat_in, *zsets[i]))
    jax.block_until_ready(outs)
    tp = (time.perf_counter() - t0) * 1e9 / (npipe - 1)
    times.append(tp)
    out = np.empty((B, S, D), dtype=np.float32)
    full = np.asarray(out_arrs[out_names.index("out")]).reshape(n_cores, SL, D)
    for c in range(n_cores):
        gi, p = divmod(c, PG)
        out[gi, p * SL : (p + 1) * SL, :] = full[c]
    return out, times


def _dispatch_floor(iters=5):
    """Measure the axon dispatch floor with a trivial jitted op on all 8 devices."""
    import time

    import jax
    import jax.numpy as jnp
    from jax.sharding import Mesh, PartitionSpec, NamedSharding

    devices = jax.devices()[:NCORES]
    mesh = Mesh(np.asarray(devices), ("core",))
    shd = NamedSharding(mesh, PartitionSpec("core"))
    x = jax.device_put(np.ones((NCORES, 8), np.float32), shd)
    f = jax.jit(lambda a: a + 1.0)
    jax.block_until_ready(f(x))
    times = []
    for _ in range(iters):
        t0 = time.perf_counter()
        jax.block_until_ready(f(x))
        times.append((time.perf_counter() - t0) * 1e9)
    return times


# revision 31
# speedup vs baseline: 2.7906x; 1.1595x over previous
"""Trainium2 Bass kernel for nn_MultiHeadAttention (B=2, S=2048, D=1024, H=16, dk=dv=64).

Sharding: 8 cores = 2 batch groups x 4 cores. Core c (g=c//4, p=c%4) computes
attention for 4 heads (heads p*4..p*4+3) of batch g over all 2048 tokens, then
an 8-rank AllToAll redistributes the attention output so core c holds all 16
heads for its 512-token slice; fc + residual + LayerNorm run token-parallel.

Layout choices (all matmul contractions live on the partition axis):
  - host supplies q/k/v transposed (XT[c, i]) so projections need no on-device
    transpose: QhT/KhT = W.T-free (lhsT=W tiles), Vh token-major (lhsT=XT tiles)
  - scores are computed transposed, ST[j, i] (keys on partitions), two heads at
    a time via row-tiled K=64 matmuls; exp runs on ACT with scale=1/sqrt(dk)
  - attention output accumulates as OT[d, i] via col-tiled M=64 matmuls
    (lhsT = Vh), softmax denominators ride along as col-tiled M=1 ones-matmuls
  - normalization: r=1/sums broadcast across partitions with a K=1 ones-matmul,
    then one DVE multiply
  - fc contracts over an extended 2048-row axis (both groups' AllToAll blocks);
    the host zeroes the other group's rows of w_fc so no per-rank addressing is
    needed (the program is identical on all 8 cores).
"""

import os
import sys

import numpy as np

if "/opt/trn_rl_repo" not in sys.path:
    sys.path.insert(0, "/opt/trn_rl_repo")

B, S, D = 2, 2048, 1024
H, DK, DV = 16, 64, 64
LN_EPS = 1e-5

NCORES = 8
PG = 4          # cores per batch group
HPC = 4         # heads per core
DPC = HPC * DV  # 256 local output dims per core
SL = S // PG    # 512 tokens per core for fc/LN
NC_TILES = D // 128  # 8 contraction tiles for projections

_CACHE = {}


def _build(trivial_ln: bool, debug: bool = False):
    import concourse.bass as bass  # noqa: F401
    import concourse.mybir as mybir
    import concourse.tile as tile
    from concourse import bacc

    f32 = mybir.dt.float32
    bf16 = mybir.dt.bfloat16
    AF = mybir.ActivationFunctionType
    OP = mybir.AluOpType

    nc = bacc.Bacc()

    xt_q = nc.dram_tensor("xt_q", [D, S], bf16, kind="ExternalInput")
    xt_k = nc.dram_tensor("xt_k", [D, S], bf16, kind="ExternalInput")
    xt_v = nc.dram_tensor("xt_v", [D, S], bf16, kind="ExternalInput")
    wq_d = nc.dram_tensor("wq", [D, DPC], bf16, kind="ExternalInput")
    wk_d = nc.dram_tensor("wk", [D, DPC], bf16, kind="ExternalInput")
    wv_d = nc.dram_tensor("wv", [D, DPC], bf16, kind="ExternalInput")
    wfcx_d = nc.dram_tensor("wfcx", [2 * D, D], bf16, kind="ExternalInput")
    resid_d = nc.dram_tensor("resid", [SL, D], f32, kind="ExternalInput")
    gamma_d = nc.dram_tensor("gamma", [1, D], f32, kind="ExternalInput")
    beta_d = nc.dram_tensor("beta", [1, D], f32, kind="ExternalInput")
    out_d = nc.dram_tensor("out", [SL, D], f32, kind="ExternalOutput")
    dbg = {}
    if debug:
        bf16_ = mybir.dt.bfloat16
        dbg["qht"] = nc.dram_tensor("dbg_qht", [2, 128, S], bf16_, kind="ExternalOutput")
        dbg["kht"] = nc.dram_tensor("dbg_kht", [2, 128, S], bf16_, kind="ExternalOutput")
        dbg["vh"] = nc.dram_tensor("dbg_vh", [16, 128, DPC], bf16_, kind="ExternalOutput")
        dbg["ex"] = nc.dram_tensor("dbg_ex", [128, 1024], bf16_, kind="ExternalOutput")
        dbg["ot"] = nc.dram_tensor("dbg_ot", [128, 512], f32, kind="ExternalOutput")
        dbg["sums"] = nc.dram_tensor("dbg_sums", [1, 1024], f32, kind="ExternalOutput")
        dbg["osc"] = nc.dram_tensor("dbg_osc", [128, 512], bf16_, kind="ExternalOutput")
        dbg["ogout"] = nc.dram_tensor(
            "dbg_ogout", [NCORES, DPC, SL], bf16_, kind="ExternalOutput"
        )
        dbg["r"] = nc.dram_tensor("dbg_r", [1, 1024], f32, kind="ExternalOutput")
        dbg["bc"] = nc.dram_tensor("dbg_bc", [128, 512], f32, kind="ExternalOutput")

    with tile.TileContext(nc) as tc:
        with (
            tc.tile_pool(name="consts", bufs=1) as consts,
            tc.tile_pool(name="persist", bufs=1) as persist,
            tc.tile_pool(name="stream", bufs=3) as stream,
            tc.tile_pool(name="work", bufs=3) as work,
            tc.tile_pool(name="dram", bufs=1, space="DRAM") as dram,
        ):
            ones_sb = consts.tile([128, 128], f32, tag="ones", name="ones_sb")
            nc.vector.memset(ones_sb[:], 1.0)
            ones_bf = consts.tile([128, 128], bf16, tag="onesbf", name="ones_bf")
            nc.vector.memset(ones_bf[:], 1.0)
            eps_sb = consts.tile([128, 1], f32, tag="eps", name="eps_sb")
            nc.vector.memset(eps_sb[:], LN_EPS)

            # ---- weights for the projections
            wq_sb, wk_sb, wv_sb = [], [], []
            for ci in range(NC_TILES):
                for lst, dsrc, nm in (
                    (wq_sb, wq_d, "wq"),
                    (wk_sb, wk_d, "wk"),
                    (wv_sb, wv_d, "wv"),
                ):
                    t = persist.tile([128, DPC], bf16, tag=f"{nm}{ci}", name=f"{nm}{ci}_sb")
                    nc.sync.dma_start(out=t[:], in_=dsrc[ci * 128 : (ci + 1) * 128, :])
                    lst.append(t)

            # ---- gamma/beta broadcast tiles (general LN path only)
            gbc_sb = bbc_sb = None
            if not trivial_ln:
                gam_row = consts.tile([1, D], f32, tag="gam_row", name="gam_row")
                nc.sync.dma_start(out=gam_row[:], in_=gamma_d[:])
                bet_row = consts.tile([1, D], f32, tag="bet_row", name="bet_row")
                nc.sync.dma_start(out=bet_row[:], in_=beta_d[:])
                gbc_sb = consts.tile([128, D], f32, tag="gbc", name="gbc_sb")
                bbc_sb = consts.tile([128, D], f32, tag="bbc", name="bbc_sb")
                with tc.tile_pool(name="ps_setup", bufs=1, space="PSUM") as ps_setup:
                    for row, dst, nm in ((gam_row, gbc_sb, "g"), (bet_row, bbc_sb, "b")):
                        bc_ps = ps_setup.tile([128, D], f32, tag="lnbc", name=f"lnbc_{nm}")
                        for e in range(2):
                            nc.tensor.matmul(
                                bc_ps[:, e * 512 : (e + 1) * 512],
                                ones_sb[0:1, :],
                                row[:, e * 512 : (e + 1) * 512],
                                start=True,
                                stop=True,
                            )
                        nc.vector.tensor_copy(out=dst[:], in_=bc_ps[:])

            # ---- projections ----------------------------------------------
            # QhT/KhT: [256(d), 2048(i)] as two [128, 2048] tiles (d on partitions,
            # head pair per tile). Vh: 16 token tiles [128(j), 256(d)].
            qht_sb = [
                persist.tile([128, S], bf16, tag=f"qht{t}", name=f"qht{t}") for t in range(2)
            ]
            kht_sb = [
                persist.tile([128, S], bf16, tag=f"kht{t}", name=f"kht{t}") for t in range(2)
            ]
            vh_sb = [
                persist.tile([128, DPC], bf16, tag=f"vh{j}", name=f"vh{j}") for j in range(16)
            ]

            with tc.tile_pool(name="ps_proj", bufs=8, space="PSUM") as ps_proj:
                for which, wsb, dsrc, dst in (
                    ("k", wk_sb, xt_k, kht_sb),
                    ("q", wq_sb, xt_q, qht_sb),
                ):
                    ps = [
                        ps_proj.tile([128, 512], f32, tag="proj", name=f"ps_{which}{i}")
                        for i in range(8)
                    ]
                    for ci in range(NC_TILES):
                        xt_t = stream.tile([128, S], bf16, tag="xt", name=f"xt_{which}{ci}")
                        nc.sync.dma_start(out=xt_t[:], in_=dsrc[ci * 128 : (ci + 1) * 128, :])
                        for t in range(2):
                            for s4 in range(4):
                                nc.tensor.matmul(
                                    ps[t * 4 + s4][:],
                                    wsb[ci][:, t * 128 : (t + 1) * 128],
                                    xt_t[:, s4 * 512 : (s4 + 1) * 512],
                                    start=(ci == 0),
                                    stop=(ci == NC_TILES - 1),
                                )
                    for t in range(2):
                        for s4 in range(4):
                            nc.vector.tensor_copy(
                                out=dst[t][:, s4 * 512 : (s4 + 1) * 512],
                                in_=ps[t * 4 + s4][:],
                            )

                # V: one accumulation group per PSUM bank (two interleaved
                # groups in one bank break: a later group's start= wipes the
                # earlier group's partials on the shared partitions). bf16
                # xt_v tiles are small enough to keep resident; loop it-outer.
                xtv_t = []
                for ci in range(NC_TILES):
                    t = stream.tile([128, S], bf16, tag=f"xtv{ci}", bufs=1, name=f"xt_v{ci}")
                    nc.sync.dma_start(out=t[:], in_=xt_v[ci * 128 : (ci + 1) * 128, :])
                    xtv_t.append(t)
                for it in range(16):
                    vps = ps_proj.tile([128, 512], f32, tag="proj", name=f"ps_v{it}")
                    for ci in range(NC_TILES):
                        nc.tensor.matmul(
                            vps[:, 0:DPC],
                            xtv_t[ci][:, it * 128 : (it + 1) * 128],
                            wv_sb[ci][:],
                            start=(ci == 0),
                            stop=(ci == NC_TILES - 1),
                        )
                    nc.vector.tensor_copy(out=vh_sb[it][:], in_=vps[:, 0:DPC])

            if debug:
                for t in range(2):
                    nc.sync.dma_start(out=dbg["qht"][t], in_=qht_sb[t][:])
                    nc.sync.dma_start(out=dbg["kht"][t], in_=kht_sb[t][:])
                for j in range(16):
                    nc.sync.dma_start(out=dbg["vh"][j], in_=vh_sb[j][:])

            # ---- attention -------------------------------------------------
            # og{pair}_in[dest_rank, d_pair_local(128), i_slab]: our scaled OT
            # block for each destination (duplicated across the two batch
            # groups). One AllToAll per head pair so the first exchange
            # overlaps the second pair's compute.
            og_in = [
                dram.tile([NCORES, 128, SL], bf16, tag=f"og_in{p}", name=f"og_in{p}")
                for p in range(2)
            ]
            og_out = [
                dram.tile([NCORES, 128, SL], bf16, tag=f"og_out{p}", name=f"og_out{p}")
                for p in range(2)
            ]

            # fc weights + residual: load early so the DMAs overlap attention
            wfx_sb = []
            for t in range(16):
                w = persist.tile([128, D], bf16, tag=f"wfx{t}", name=f"wfx{t}")
                nc.sync.dma_start(out=w[:], in_=wfcx_d[t * 128 : (t + 1) * 128, :])
                wfx_sb.append(w)
            res_sb = []
            for it in range(4):
                r = persist.tile([128, D], f32, tag=f"res{it}", name=f"res{it}")
                nc.sync.dma_start(out=r[:], in_=resid_d[it * 128 : (it + 1) * 128, :])
                res_sb.append(r)

            with tc.tile_pool(name="ps_attn", bufs=1, space="PSUM") as ps_attn:
                for pair in range(2):  # head pair (2 heads each)
                    for s in range(4):  # 512-query slab
                        ot_ps = ps_attn.tile(
                            [128, 512], f32, tag="ot", bufs=2, name=f"ot{pair}{s}"
                        )
                        # both heads' key-sums at partition 0 (bank per head):
                        # custom-DVE reciprocal misbehaves on partition-base!=0
                        sums_ps = ps_attn.tile(
                            [1, 1024], f32, tag="sums", bufs=1, name=f"sums{pair}{s}"
                        )
                        ex_tiles = {}
                        for jj in range(17):  # 16 key tiles, software-skewed by 1
                            if jj < 16:
                                sc = ps_attn.tile(
                                    [128, 1024], f32, tag="sc", bufs=2, name=f"sc{pair}{s}{jj}"
                                )
                                for hi in range(2):
                                    nc.tensor.matmul(
                                        sc[:, hi * 512 : (hi + 1) * 512],
                                        kht_sb[pair][
                                            hi * 64 : (hi + 1) * 64, jj * 128 : (jj + 1) * 128
                                        ],
                                        qht_sb[pair][
                                            hi * 64 : (hi + 1) * 64, s * 512 : (s + 1) * 512
                                        ],
                                        start=True,
                                        stop=True,
                                    )
                                ex = work.tile(
                                    [128, 1024], bf16, tag="ex", bufs=3, name=f"ex{pair}{s}{jj}"
                                )
                                nc.scalar.activation(
                                    out=ex[:], in_=sc[:], func=AF.Exp, scale=1.0 / (DK**0.5)
                                )
                                if debug and pair == 0 and s == 0 and jj == 0:
                                    nc.sync.dma_start(out=dbg["ex"][:], in_=ex[:])
                                ex_tiles[jj] = ex
                            if jj >= 1:
                                j0 = jj - 1
                                ex = ex_tiles.pop(j0)
                                for hi in range(2):
                                    nc.tensor.matmul(
                                        ot_ps[hi * 64 : (hi + 1) * 64, :],
                                        vh_sb[j0][:, pair * 128 + hi * 64 : pair * 128 + (hi + 1) * 64],
                                        ex[:, hi * 512 : (hi + 1) * 512],
                                        start=(j0 == 0),
                                        stop=(j0 == 15),
                                    )
                                for hi in range(2):
                                    nc.tensor.matmul(
                                        sums_ps[0:1, hi * 512 : (hi + 1) * 512],
                                        ones_bf[:, 0:1],
                                        ex[:, hi * 512 : (hi + 1) * 512],
                                        start=(j0 == 0),
                                        stop=(j0 == 15),
                                    )

                        # softmax normalization: r = 1/sums, broadcast over
                        # partitions via a K=1 ones-matmul, multiply into OT.
                        r_sb = work.tile([1, 1024], f32, tag="r", bufs=2, name=f"r{pair}{s}")
                        r_scr = work.tile([1, 1024], f32, tag="rscr", bufs=2, name=f"rs{pair}{s}")
                        nc.vector.reciprocal_approx_accurate(
                            out=r_sb[:], in_=sums_ps[:], scratch=r_scr[:]
                        )
                        bc_ps = ps_attn.tile(
                            [128, 1024], f32, tag="sc", bufs=2, name=f"bc{pair}{s}"
                        )
                        for hi in range(2):
                            nc.tensor.matmul(
                                bc_ps[hi * 64 : (hi + 1) * 64, 0:512],
                                ones_sb[0:1, 0:64],
                                r_sb[0:1, hi * 512 : (hi + 1) * 512],
                                start=True,
                                stop=True,
                            )
                        bc_sb = work.tile([128, 512], f32, tag="bc", bufs=2, name=f"bc_sb{pair}{s}")
                        nc.vector.tensor_copy(out=bc_sb[:], in_=bc_ps[:, 0:512])
                        osc = work.tile([128, 512], bf16, tag="osc", bufs=2, name=f"osc{pair}{s}")
                        nc.vector.tensor_mul(out=osc[:], in0=ot_ps[:], in1=bc_sb[:])
                        if debug and pair == 0 and s == 0:
                            nc.sync.dma_start(out=dbg["r"][:], in_=r_sb[:])
                            nc.sync.dma_start(out=dbg["bc"][:], in_=bc_sb[:])
                            dot = work.tile([128, 512], f32, tag="dbgot", name="dbg_ot_sb")
                            nc.vector.tensor_copy(out=dot[:], in_=ot_ps[:])
                            nc.sync.dma_start(out=dbg["ot"][:], in_=dot[:])
                            dsm = work.tile([1, 1024], f32, tag="dbgsm", name="dbg_sm_sb")
                            nc.vector.tensor_copy(out=dsm[:], in_=sums_ps[:])
                            nc.sync.dma_start(out=dbg["sums"][:], in_=dsm[:])
                            nc.sync.dma_start(out=dbg["osc"][:], in_=osc[:])
                        for grp in range(2):
                            nc.sync.dma_start(
                                out=og_in[pair][grp * 4 + s, :, :],
                                in_=osc[:],
                            )

                    # ---- exchange for this head pair (first one overlaps the
                    # second pair's compute)
                    nc.gpsimd.collective_compute(
                        "AllToAll",
                        OP.bypass,
                        replica_groups=[list(range(NCORES))],
                        ins=[og_in[pair].opt()],
                        outs=[og_out[pair].opt()],
                    )

            if debug:
                for p in range(2):
                    nc.sync.dma_start(
                        out=dbg["ogout"][:, p * 128 : (p + 1) * 128, :], in_=og_out[p][:]
                    )

            # ---- fc + residual + LayerNorm ---------------------------------
            with tc.tile_pool(name="ps_fc", bufs=1, space="PSUM") as ps_fc:
                fc_ps = [
                    ps_fc.tile([128, 512], f32, tag="fc", bufs=8, name=f"fc{i}")
                    for i in range(8)
                ]
                for t in range(16):  # extended contraction: (rank r, head pair)
                    otx = stream.tile([128, 512], bf16, tag="otx", bufs=4, name=f"otx{t}")
                    nc.sync.dma_start(out=otx[:], in_=og_out[t % 2][t // 2, :, :])
                    wfx = wfx_sb[(t // 2) * 2 + (t % 2)]
                    for it in range(4):
                        for e in range(2):
                            nc.tensor.matmul(
                                fc_ps[it * 2 + e][:],
                                otx[:, it * 128 : (it + 1) * 128],
                                wfx[:, e * 512 : (e + 1) * 512],
                                start=(t == 0),
                                stop=(t == 15),
                            )

                for it in range(4):
                    res_t = res_sb[it]
                    y = work.tile([128, D], f32, tag="y", bufs=2, name=f"y{it}")
                    for e in range(2):
                        nc.vector.tensor_add(
                            out=y[:, e * 512 : (e + 1) * 512],
                            in0=fc_ps[it * 2 + e][:],
                            in1=res_t[:, e * 512 : (e + 1) * 512],
                        )
                    st = work.tile([128, 2, 6], f32, tag="st", bufs=2, name=f"st{it}")
                    for e in range(2):
                        nc.vector.bn_stats(out=st[:, e, :], in_=y[:, e * 512 : (e + 1) * 512])
                    mv = work.tile([128, 2], f32, tag="mv", bufs=2, name=f"mv{it}")
                    nc.vector.bn_aggr(out=mv[:], in_=st[:])
                    # rstd = 1/sqrt(var+eps); nmr = -mu*rstd
                    sd = work.tile([128, 1], f32, tag="sd", bufs=2, name=f"sd{it}")
                    nc.scalar.activation(
                        out=sd[:], in_=mv[:, 1:2], func=AF.Sqrt, bias=eps_sb[:], scale=1.0
                    )
                    rstd = work.tile([128, 1], f32, tag="rstd", bufs=2, name=f"rstd{it}")
                    nc.vector.reciprocal(out=rstd[:], in_=sd[:])
                    nmr = work.tile([128, 1], f32, tag="nmr", bufs=2, name=f"nmr{it}")
                    nc.vector.tensor_scalar(
                        out=nmr[:],
                        in0=mv[:, 0:1],
                        scalar1=rstd[:],
                        scalar2=-1.0,
                        op0=OP.mult,
                        op1=OP.mult,
                    )
                    z = work.tile([128, D], f32, tag="z", bufs=2, name=f"z{it}")
                    nc.vector.tensor_scalar(
                        out=z[:],
                        in0=y[:],
                        scalar1=rstd[:],
                        scalar2=nmr[:],
                        op0=OP.mult,
                        op1=OP.add,
                    )
                    if not trivial_ln:
                        z2 = work.tile([128, D], f32, tag="z2", bufs=2, name=f"z2{it}")
                        nc.vector.tensor_mul(out=z2[:], in0=z[:], in1=gbc_sb[:])
                        z3 = work.tile([128, D], f32, tag="z3", bufs=2, name=f"z3{it}")
                        nc.vector.tensor_add(out=z3[:], in0=z2[:], in1=bbc_sb[:])
                        z = z3
                    nc.sync.dma_start(out=out_d[it * 128 : (it + 1) * 128, :], in_=z[:])

    nc.compile()
    return nc


def _get_nc(trivial_ln: bool, debug: bool = False):
    key = ("nc", trivial_ln, debug)
    if key not in _CACHE:
        _CACHE[key] = _build(trivial_ln, debug)
    return _CACHE[key]


def _shard(inputs):
    import ml_dtypes

    bf = ml_dtypes.bfloat16
    q = np.ascontiguousarray(np.asarray(inputs["q"], dtype=np.float32))
    k = np.ascontiguousarray(np.asarray(inputs["k"], dtype=np.float32))
    v = np.ascontiguousarray(np.asarray(inputs["v"], dtype=np.float32))
    w_q = np.asarray(inputs["w_q"], dtype=np.float32)
    w_k = np.asarray(inputs["w_k"], dtype=np.float32)
    w_v = np.asarray(inputs["w_v"], dtype=np.float32)
    w_fc = np.asarray(inputs["w_fc"], dtype=np.float32)
    gamma = np.asarray(inputs["ln_gamma"], dtype=np.float32).reshape(1, D)
    beta = np.asarray(inputs["ln_beta"], dtype=np.float32).reshape(1, D)

    xt = {}
    for gi in range(2):
        xt[gi] = (
            np.ascontiguousarray(q[gi].T).astype(bf),
            np.ascontiguousarray(k[gi].T).astype(bf),
            np.ascontiguousarray(v[gi].T).astype(bf),
        )
    wq_s = [np.ascontiguousarray(w_q[:, p * DPC : (p + 1) * DPC]).astype(bf) for p in range(PG)]
    wk_s = [np.ascontiguousarray(w_k[:, p * DPC : (p + 1) * DPC]).astype(bf) for p in range(PG)]
    wv_s = [np.ascontiguousarray(w_v[:, p * DPC : (p + 1) * DPC]).astype(bf) for p in range(PG)]
    # extended fc weights: row block r (256 rows) = w_fc rows for rank r's heads,
    # zeroed when rank r belongs to the other batch group
    wfcx = []
    for gi in range(2):
        w = np.zeros((2 * D, D), dtype=np.float32)
        for r in range(NCORES):
            if r // PG == gi:
                hp = r % PG
                w[r * DPC : (r + 1) * DPC, :] = w_fc[hp * DPC : (hp + 1) * DPC, :]
        wfcx.append(w.astype(bf))

    in_maps = []
    for c in range(NCORES):
        gi, p = divmod(c, PG)
        in_maps.append(
            {
                "xt_q": xt[gi][0],
                "xt_k": xt[gi][1],
                "xt_v": xt[gi][2],
                "wq": wq_s[p],
                "wk": wk_s[p],
                "wv": wv_s[p],
                "wfcx": wfcx[gi],
                "resid": np.ascontiguousarray(q[gi, p * SL : (p + 1) * SL, :]),
                "gamma": gamma,
                "beta": beta,
            }
        )
    trivial_ln = bool(np.all(gamma == 1.0) and np.all(beta == 0.0))
    return in_maps, trivial_ln


def _run(inputs, trace=False, debug=False):
    from concourse.bass_utils import run_bass_kernel_spmd

    in_maps, trivial_ln = _shard(inputs)
    nc = _get_nc(trivial_ln, debug)
    res = run_bass_kernel_spmd(
        nc, in_maps, core_ids=list(range(NCORES)), trace=trace
    )
    out = np.empty((B, S, D), dtype=np.float32)
    for c in range(NCORES):
        gi, p = divmod(c, PG)
        out[gi, p * SL : (p + 1) * SL, :] = res.results[c]["out"]
    return out, res


def kernel(**inputs) -> np.ndarray:
    out, _ = _run(inputs)
    return out


def _timed_exec(inputs, iters=5):
    """Execute on 8 cores with device-resident inputs; return (out, [dt_ns])."""
    import time

    import jax
    import jax.numpy as jnp  # noqa: F401
    from jax.sharding import Mesh, PartitionSpec
    from jax.experimental.shard_map import shard_map

    import concourse.mybir as mybir
    from concourse import bass2jax

    in_maps, trivial_ln = _shard(inputs)
    nc = _get_nc(trivial_ln)
    bass2jax.install_neuronx_cc_hook()

    n_cores = NCORES
    partition_name = nc.partition_id_tensor.name if nc.partition_id_tensor else None
    in_names, out_names, out_avals, zero_outs = [], [], [], []
    for alloc in nc.m.functions[0].allocations:
        if not isinstance(alloc, mybir.MemoryLocationSet):
            continue
        name = alloc.memorylocations[0].name
        if alloc.kind == "ExternalInput":
            if name != partition_name:
                in_names.append(name)
        elif alloc.kind == "ExternalOutput":
            shape = tuple(alloc.tensor_shape)
            dtype = mybir.dt.np(alloc.dtype)
            out_names.append(name)
            out_avals.append(jax.core.ShapedArray(shape, dtype))
            zero_outs.append(np.zeros(shape, dtype))
    n_params = len(in_names)
    n_outs = len(out_avals)
    all_names = in_names + out_names
    if partition_name is not None:
        all_names = all_names + [partition_name]
    donate = tuple(range(n_params, n_params + n_outs))

    def _body(*args):
        operands = list(args)
        if partition_name is not None:
            operands.append(bass2jax.partition_id_tensor())
        outs = bass2jax._bass_exec_p.bind(
            *operands,
            out_avals=tuple(out_avals),
            in_names=tuple(all_names),
            out_names=tuple(out_names),
            lowering_input_output_aliases=(),
            sim_require_finite=True,
            sim_require_nnan=True,
            nc=nc,
        )
        return tuple(outs)

    devices = jax.devices()[:n_cores]
    mesh = Mesh(np.asarray(devices), ("core",))
    in_specs = (PartitionSpec("core"),) * (n_params + n_outs)
    out_specs = (PartitionSpec("core"),) * n_outs
    sharded = jax.jit(
        shard_map(_body, mesh=mesh, in_specs=in_specs, out_specs=out_specs, check_rep=False),
        donate_argnums=donate,
        keep_unused=True,
    )
    from jax.sharding import NamedSharding

    shd = NamedSharding(mesh, PartitionSpec("core"))
    concat_in = [
        jax.device_put(
            np.concatenate([np.asarray(in_maps[c][n]) for c in range(n_cores)], axis=0), shd
        )
        for n in in_names
    ]
    times = []
    out_arrs = None
    for _ in range(iters):
        zeros_dev = [
            jax.device_put(np.zeros((n_cores * z.shape[0], *z.shape[1:]), z.dtype), shd)
            for z in zero_outs
        ]
        jax.block_until_ready(zeros_dev)
        t0 = time.perf_counter()
        out_arrs = sharded(*concat_in, *zeros_dev)
        jax.block_until_ready(out_arrs)
        times.append((time.perf_counter() - t0) * 1e9)

    # pipelined: many dispatches in flight so the axon RPC overhead overlaps;
    # marginal per-call time approximates device execution time
    npipe = 16
    zsets = [
        [
            jax.device_put(np.zeros((n_cores * z.shape[0], *z.shape[1:]), z.dtype), shd)
            for z in zero_outs
        ]
        for _ in range(npipe)
    ]
    jax.block_until_ready(zsets)
    outs = [sharded(*concat_in, *zsets[0])]
    jax.block_until_ready(outs[0])  # warm
    t0 = time.perf_counter()
    for i in range(1, npipe):
        outs.append(sharded(*concat_in, *zsets[i]))
    jax.block_until_ready(outs)
    tp = (time.perf_counter() - t0) * 1e9 / (npipe - 1)
    times.append(tp)
    out = np.empty((B, S, D), dtype=np.float32)
    full = np.asarray(out_arrs[out_names.index("out")]).reshape(n_cores, SL, D)
    for c in range(n_cores):
        gi, p = divmod(c, PG)
        out[gi, p * SL : (p + 1) * SL, :] = full[c]
    return out, times


def _dispatch_floor(iters=5):
    """Measure the axon dispatch floor with a trivial jitted op on all 8 devices."""
    import time

    import jax
    import jax.numpy as jnp
    from jax.sharding import Mesh, PartitionSpec, NamedSharding

    devices = jax.devices()[:NCORES]
    mesh = Mesh(np.asarray(devices), ("core",))
    shd = NamedSharding(mesh, PartitionSpec("core"))
    x = jax.device_put(np.ones((NCORES, 8), np.float32), shd)
    f = jax.jit(lambda a: a + 1.0)
    jax.block_until_ready(f(x))
    times = []
    for _ in range(iters):
        t0 = time.perf_counter()
        jax.block_until_ready(f(x))
        times.append((time.perf_counter() - t0) * 1e9)
    return times


# revision 32
# speedup vs baseline: 2.9992x; 1.0747x over previous
"""Trainium2 Bass kernel for nn_MultiHeadAttention (B=2, S=2048, D=1024, H=16, dk=dv=64).

Sharding: 8 cores = 2 batch groups x 4 cores. Core c (g=c//4, p=c%4) computes
attention for 4 heads (heads p*4..p*4+3) of batch g over all 2048 tokens, then
an 8-rank AllToAll redistributes the attention output so core c holds all 16
heads for its 512-token slice; fc + residual + LayerNorm run token-parallel.

Layout choices (all matmul contractions live on the partition axis):
  - host supplies q/k/v transposed (XT[c, i]) so projections need no on-device
    transpose: QhT/KhT = W.T-free (lhsT=W tiles), Vh token-major (lhsT=XT tiles)
  - scores are computed transposed, ST[j, i] (keys on partitions), two heads at
    a time via row-tiled K=64 matmuls; exp runs on ACT with scale=1/sqrt(dk)
  - attention output accumulates as OT[d, i] via col-tiled M=64 matmuls
    (lhsT = Vh), softmax denominators ride along as col-tiled M=1 ones-matmuls
  - normalization: r=1/sums broadcast across partitions with a K=1 ones-matmul,
    then one DVE multiply
  - fc contracts over an extended 2048-row axis (both groups' AllToAll blocks);
    the host zeroes the other group's rows of w_fc so no per-rank addressing is
    needed (the program is identical on all 8 cores).
"""

import os
import sys

import numpy as np

if "/opt/trn_rl_repo" not in sys.path:
    sys.path.insert(0, "/opt/trn_rl_repo")

B, S, D = 2, 2048, 1024
H, DK, DV = 16, 64, 64
LN_EPS = 1e-5

NCORES = 8
PG = 4          # cores per batch group
HPC = 4         # heads per core
DPC = HPC * DV  # 256 local output dims per core
SL = S // PG    # 512 tokens per core for fc/LN
NC_TILES = D // 128  # 8 contraction tiles for projections

_CACHE = {}


def _build(trivial_ln: bool, debug: bool = False):
    import concourse.bass as bass  # noqa: F401
    import concourse.mybir as mybir
    import concourse.tile as tile
    from concourse import bacc

    f32 = mybir.dt.float32
    bf16 = mybir.dt.bfloat16
    AF = mybir.ActivationFunctionType
    OP = mybir.AluOpType

    nc = bacc.Bacc()

    xt_q = nc.dram_tensor("xt_q", [D, S], bf16, kind="ExternalInput")
    xt_k = nc.dram_tensor("xt_k", [D, S], bf16, kind="ExternalInput")
    xt_v = nc.dram_tensor("xt_v", [D, S], bf16, kind="ExternalInput")
    wq_d = nc.dram_tensor("wq", [D, DPC], bf16, kind="ExternalInput")
    wk_d = nc.dram_tensor("wk", [D, DPC], bf16, kind="ExternalInput")
    wv_d = nc.dram_tensor("wv", [D, DPC], bf16, kind="ExternalInput")
    wfcx_d = nc.dram_tensor("wfcx", [2 * D, D], bf16, kind="ExternalInput")
    resid_d = nc.dram_tensor("resid", [SL, D], f32, kind="ExternalInput")
    gamma_d = nc.dram_tensor("gamma", [1, D], f32, kind="ExternalInput")
    beta_d = nc.dram_tensor("beta", [1, D], f32, kind="ExternalInput")
    out_d = nc.dram_tensor("out", [SL, D], f32, kind="ExternalOutput")
    dbg = {}
    if debug:
        bf16_ = mybir.dt.bfloat16
        dbg["qht"] = nc.dram_tensor("dbg_qht", [2, 128, S], bf16_, kind="ExternalOutput")
        dbg["kht"] = nc.dram_tensor("dbg_kht", [2, 128, S], bf16_, kind="ExternalOutput")
        dbg["vh"] = nc.dram_tensor("dbg_vh", [16, 128, DPC], bf16_, kind="ExternalOutput")
        dbg["ex"] = nc.dram_tensor("dbg_ex", [128, 1024], bf16_, kind="ExternalOutput")
        dbg["ot"] = nc.dram_tensor("dbg_ot", [128, 512], f32, kind="ExternalOutput")
        dbg["sums"] = nc.dram_tensor("dbg_sums", [1, 1024], f32, kind="ExternalOutput")
        dbg["osc"] = nc.dram_tensor("dbg_osc", [128, 512], bf16_, kind="ExternalOutput")
        dbg["ogout"] = nc.dram_tensor(
            "dbg_ogout", [NCORES, DPC, SL], bf16_, kind="ExternalOutput"
        )
        dbg["r"] = nc.dram_tensor("dbg_r", [1, 1024], f32, kind="ExternalOutput")
        dbg["bc"] = nc.dram_tensor("dbg_bc", [128, 512], f32, kind="ExternalOutput")

    with tile.TileContext(nc) as tc:
        with (
            tc.tile_pool(name="consts", bufs=1) as consts,
            tc.tile_pool(name="persist", bufs=1) as persist,
            tc.tile_pool(name="stream", bufs=4) as stream,
            tc.tile_pool(name="work", bufs=3) as work,
            tc.tile_pool(name="dram", bufs=1, space="DRAM") as dram,
        ):
            ones_sb = consts.tile([128, 128], f32, tag="ones", name="ones_sb")
            nc.vector.memset(ones_sb[:], 1.0)
            ones_bf = consts.tile([128, 128], bf16, tag="onesbf", name="ones_bf")
            nc.vector.memset(ones_bf[:], 1.0)
            eps_sb = consts.tile([128, 1], f32, tag="eps", name="eps_sb")
            nc.vector.memset(eps_sb[:], LN_EPS)

            # ---- weights for the projections
            wq_sb, wk_sb, wv_sb = [], [], []
            for ci in range(NC_TILES):
                for lst, dsrc, nm in (
                    (wq_sb, wq_d, "wq"),
                    (wk_sb, wk_d, "wk"),
                    (wv_sb, wv_d, "wv"),
                ):
                    t = persist.tile([128, DPC], bf16, tag=f"{nm}{ci}", name=f"{nm}{ci}_sb")
                    nc.sync.dma_start(out=t[:], in_=dsrc[ci * 128 : (ci + 1) * 128, :])
                    lst.append(t)

            # ---- gamma/beta broadcast tiles (general LN path only)
            gbc_sb = bbc_sb = None
            if not trivial_ln:
                gam_row = consts.tile([1, D], f32, tag="gam_row", name="gam_row")
                nc.sync.dma_start(out=gam_row[:], in_=gamma_d[:])
                bet_row = consts.tile([1, D], f32, tag="bet_row", name="bet_row")
                nc.sync.dma_start(out=bet_row[:], in_=beta_d[:])
                gbc_sb = consts.tile([128, D], f32, tag="gbc", name="gbc_sb")
                bbc_sb = consts.tile([128, D], f32, tag="bbc", name="bbc_sb")
                with tc.tile_pool(name="ps_setup", bufs=1, space="PSUM") as ps_setup:
                    for row, dst, nm in ((gam_row, gbc_sb, "g"), (bet_row, bbc_sb, "b")):
                        bc_ps = ps_setup.tile([128, D], f32, tag="lnbc", name=f"lnbc_{nm}")
                        for e in range(2):
                            nc.tensor.matmul(
                                bc_ps[:, e * 512 : (e + 1) * 512],
                                ones_sb[0:1, :],
                                row[:, e * 512 : (e + 1) * 512],
                                start=True,
                                stop=True,
                            )
                        nc.vector.tensor_copy(out=dst[:], in_=bc_ps[:])

            # ---- projections ----------------------------------------------
            # QhT/KhT: [256(d), 2048(i)] as two [128, 2048] tiles (d on partitions,
            # head pair per tile). Vh: 16 token tiles [128(j), 256(d)].
            qht_sb = [
                persist.tile([128, S], bf16, tag=f"qht{t}", name=f"qht{t}") for t in range(2)
            ]
            kht_sb = [
                persist.tile([128, S], bf16, tag=f"kht{t}", name=f"kht{t}") for t in range(2)
            ]
            vh_sb = [
                persist.tile([128, DPC], bf16, tag=f"vh{j}", name=f"vh{j}") for j in range(16)
            ]

            with tc.tile_pool(name="ps_proj", bufs=8, space="PSUM") as ps_proj:
                for which, wsb, dsrc, dst in (
                    ("k", wk_sb, xt_k, kht_sb),
                    ("q", wq_sb, xt_q, qht_sb),
                ):
                    ps = [
                        ps_proj.tile([128, 512], f32, tag="proj", name=f"ps_{which}{i}")
                        for i in range(8)
                    ]
                    for ci in range(NC_TILES):
                        xt_t = stream.tile([128, S], bf16, tag="xt", name=f"xt_{which}{ci}")
                        nc.sync.dma_start(out=xt_t[:], in_=dsrc[ci * 128 : (ci + 1) * 128, :])
                        for t in range(2):
                            for s4 in range(4):
                                nc.tensor.matmul(
                                    ps[t * 4 + s4][:],
                                    wsb[ci][:, t * 128 : (t + 1) * 128],
                                    xt_t[:, s4 * 512 : (s4 + 1) * 512],
                                    start=(ci == 0),
                                    stop=(ci == NC_TILES - 1),
                                )
                    for t in range(2):
                        for s4 in range(4):
                            nc.vector.tensor_copy(
                                out=dst[t][:, s4 * 512 : (s4 + 1) * 512],
                                in_=ps[t * 4 + s4][:],
                            )

                # V: one accumulation group per PSUM bank (two interleaved
                # groups in one bank break: a later group's start= wipes the
                # earlier group's partials on the shared partitions). bf16
                # xt_v tiles are small enough to keep resident; loop it-outer.
                xtv_t = []
                for ci in range(NC_TILES):
                    t = stream.tile([128, S], bf16, tag=f"xtv{ci}", bufs=1, name=f"xt_v{ci}")
                    nc.sync.dma_start(out=t[:], in_=xt_v[ci * 128 : (ci + 1) * 128, :])
                    xtv_t.append(t)
                for it in range(16):
                    vps = ps_proj.tile([128, 512], f32, tag="proj", name=f"ps_v{it}")
                    for ci in range(NC_TILES):
                        nc.tensor.matmul(
                            vps[:, 0:DPC],
                            xtv_t[ci][:, it * 128 : (it + 1) * 128],
                            wv_sb[ci][:],
                            start=(ci == 0),
                            stop=(ci == NC_TILES - 1),
                        )
                    nc.vector.tensor_copy(out=vh_sb[it][:], in_=vps[:, 0:DPC])

            if debug:
                for t in range(2):
                    nc.sync.dma_start(out=dbg["qht"][t], in_=qht_sb[t][:])
                    nc.sync.dma_start(out=dbg["kht"][t], in_=kht_sb[t][:])
                for j in range(16):
                    nc.sync.dma_start(out=dbg["vh"][j], in_=vh_sb[j][:])

            # ---- attention -------------------------------------------------
            # og{pair}_in[dest_rank, d_pair_local(128), i_slab]: our scaled OT
            # block for each destination (duplicated across the two batch
            # groups). One AllToAll per head pair so the first exchange
            # overlaps the second pair's compute.
            og_in = [
                dram.tile([NCORES, 128, SL], bf16, tag=f"og_in{p}", name=f"og_in{p}")
                for p in range(2)
            ]
            og_out = [
                dram.tile([NCORES, 128, SL], bf16, tag=f"og_out{p}", name=f"og_out{p}")
                for p in range(2)
            ]

            # fc weights + residual: load early so the DMAs overlap attention
            wfx_sb = []
            for t in range(16):
                w = persist.tile([128, D], bf16, tag=f"wfx{t}", name=f"wfx{t}")
                nc.sync.dma_start(out=w[:], in_=wfcx_d[t * 128 : (t + 1) * 128, :])
                wfx_sb.append(w)
            res_sb = []
            for it in range(4):
                r = persist.tile([128, D], f32, tag=f"res{it}", name=f"res{it}")
                nc.sync.dma_start(out=r[:], in_=resid_d[it * 128 : (it + 1) * 128, :])
                res_sb.append(r)

            with tc.tile_pool(name="ps_attn", bufs=1, space="PSUM") as ps_attn:
                for pair in range(2):  # head pair (2 heads each)
                    for s in range(4):  # 512-query slab
                        ot_ps = ps_attn.tile(
                            [128, 512], f32, tag="ot", bufs=2, name=f"ot{pair}{s}"
                        )
                        # both heads' key-sums at partition 0 (bank per head):
                        # custom-DVE reciprocal misbehaves on partition-base!=0
                        sums_ps = ps_attn.tile(
                            [1, 1024], f32, tag="sums", bufs=1, name=f"sums{pair}{s}"
                        )
                        ex_tiles = {}
                        for jj in range(17):  # 16 key tiles, software-skewed by 1
                            if jj < 16:
                                sc = ps_attn.tile(
                                    [128, 1024], f32, tag="sc", bufs=2, name=f"sc{pair}{s}{jj}"
                                )
                                for hi in range(2):
                                    nc.tensor.matmul(
                                        sc[:, hi * 512 : (hi + 1) * 512],
                                        kht_sb[pair][
                                            hi * 64 : (hi + 1) * 64, jj * 128 : (jj + 1) * 128
                                        ],
                                        qht_sb[pair][
                                            hi * 64 : (hi + 1) * 64, s * 512 : (s + 1) * 512
                                        ],
                                        start=True,
                                        stop=True,
                                    )
                                ex = work.tile(
                                    [128, 1024], bf16, tag="ex", bufs=4, name=f"ex{pair}{s}{jj}"
                                )
                                nc.scalar.activation(
                                    out=ex[:], in_=sc[:], func=AF.Exp, scale=1.0 / (DK**0.5)
                                )
                                if debug and pair == 0 and s == 0 and jj == 0:
                                    nc.sync.dma_start(out=dbg["ex"][:], in_=ex[:])
                                ex_tiles[jj] = ex
                            if jj >= 1:
                                j0 = jj - 1
                                ex = ex_tiles.pop(j0)
                                for hi in range(2):
                                    nc.tensor.matmul(
                                        ot_ps[hi * 64 : (hi + 1) * 64, :],
                                        vh_sb[j0][:, pair * 128 + hi * 64 : pair * 128 + (hi + 1) * 64],
                                        ex[:, hi * 512 : (hi + 1) * 512],
                                        start=(j0 == 0),
                                        stop=(j0 == 15),
                                    )
                                for hi in range(2):
                                    nc.tensor.matmul(
                                        sums_ps[0:1, hi * 512 : (hi + 1) * 512],
                                        ones_bf[:, 0:1],
                                        ex[:, hi * 512 : (hi + 1) * 512],
                                        start=(j0 == 0),
                                        stop=(j0 == 15),
                                    )

                        # softmax normalization: r = 1/sums, broadcast over
                        # partitions via a K=1 ones-matmul, multiply into OT.
                        r_sb = work.tile([1, 1024], f32, tag="r", bufs=2, name=f"r{pair}{s}")
                        r_scr = work.tile([1, 1024], f32, tag="rscr", bufs=2, name=f"rs{pair}{s}")
                        nc.vector.reciprocal_approx_accurate(
                            out=r_sb[:], in_=sums_ps[:], scratch=r_scr[:]
                        )
                        bc_ps = ps_attn.tile(
                            [128, 512], f32, tag="ot", bufs=2, name=f"bc{pair}{s}"
                        )
                        for hi in range(2):
                            nc.tensor.matmul(
                                bc_ps[hi * 64 : (hi + 1) * 64, :],
                                ones_sb[0:1, 0:64],
                                r_sb[0:1, hi * 512 : (hi + 1) * 512],
                                start=True,
                                stop=True,
                            )
                        bc_sb = work.tile([128, 512], f32, tag="bc", bufs=2, name=f"bc_sb{pair}{s}")
                        nc.vector.tensor_copy(out=bc_sb[:], in_=bc_ps[:, 0:512])
                        osc = work.tile([128, 512], bf16, tag="osc", bufs=2, name=f"osc{pair}{s}")
                        nc.vector.tensor_mul(out=osc[:], in0=ot_ps[:], in1=bc_sb[:])
                        if debug and pair == 0 and s == 0:
                            nc.sync.dma_start(out=dbg["r"][:], in_=r_sb[:])
                            nc.sync.dma_start(out=dbg["bc"][:], in_=bc_sb[:])
                            dot = work.tile([128, 512], f32, tag="dbgot", name="dbg_ot_sb")
                            nc.vector.tensor_copy(out=dot[:], in_=ot_ps[:])
                            nc.sync.dma_start(out=dbg["ot"][:], in_=dot[:])
                            dsm = work.tile([1, 1024], f32, tag="dbgsm", name="dbg_sm_sb")
                            nc.vector.tensor_copy(out=dsm[:], in_=sums_ps[:])
                            nc.sync.dma_start(out=dbg["sums"][:], in_=dsm[:])
                            nc.sync.dma_start(out=dbg["osc"][:], in_=osc[:])
                        for grp in range(2):
                            nc.sync.dma_start(
                                out=og_in[pair][grp * 4 + s, :, :],
                                in_=osc[:],
                            )

                    # ---- exchange for this head pair (first one overlaps the
                    # second pair's compute)
                    nc.gpsimd.collective_compute(
                        "AllToAll",
                        OP.bypass,
                        replica_groups=[list(range(NCORES))],
                        ins=[og_in[pair].opt()],
                        outs=[og_out[pair].opt()],
                    )

            if debug:
                for p in range(2):
                    nc.sync.dma_start(
                        out=dbg["ogout"][:, p * 128 : (p + 1) * 128, :], in_=og_out[p][:]
                    )

            # ---- fc + residual + LayerNorm ---------------------------------
            with tc.tile_pool(name="ps_fc", bufs=1, space="PSUM") as ps_fc:
                fc_ps = [
                    ps_fc.tile([128, 512], f32, tag="fc", bufs=8, name=f"fc{i}")
                    for i in range(8)
                ]
                for t in range(16):  # extended contraction: (rank r, head pair)
                    otx = stream.tile([128, 512], bf16, tag="otx", bufs=4, name=f"otx{t}")
                    nc.sync.dma_start(out=otx[:], in_=og_out[t % 2][t // 2, :, :])
                    wfx = wfx_sb[(t // 2) * 2 + (t % 2)]
                    for it in range(4):
                        for e in range(2):
                            nc.tensor.matmul(
                                fc_ps[it * 2 + e][:],
                                otx[:, it * 128 : (it + 1) * 128],
                                wfx[:, e * 512 : (e + 1) * 512],
                                start=(t == 0),
                                stop=(t == 15),
                            )

                for it in range(4):
                    res_t = res_sb[it]
                    y = work.tile([128, D], f32, tag="y", bufs=2, name=f"y{it}")
                    for e in range(2):
                        nc.vector.tensor_add(
                            out=y[:, e * 512 : (e + 1) * 512],
                            in0=fc_ps[it * 2 + e][:],
                            in1=res_t[:, e * 512 : (e + 1) * 512],
                        )
                    st = work.tile([128, 2, 6], f32, tag="st", bufs=2, name=f"st{it}")
                    for e in range(2):
                        nc.vector.bn_stats(out=st[:, e, :], in_=y[:, e * 512 : (e + 1) * 512])
                    mv = work.tile([128, 2], f32, tag="mv", bufs=2, name=f"mv{it}")
                    nc.vector.bn_aggr(out=mv[:], in_=st[:])
                    # rstd = 1/sqrt(var+eps); nmr = -mu*rstd
                    sd = work.tile([128, 1], f32, tag="sd", bufs=2, name=f"sd{it}")
                    nc.scalar.activation(
                        out=sd[:], in_=mv[:, 1:2], func=AF.Sqrt, bias=eps_sb[:], scale=1.0
                    )
                    rstd = work.tile([128, 1], f32, tag="rstd", bufs=2, name=f"rstd{it}")
                    nc.vector.reciprocal(out=rstd[:], in_=sd[:])
                    nmr = work.tile([128, 1], f32, tag="nmr", bufs=2, name=f"nmr{it}")
                    nc.vector.tensor_scalar(
                        out=nmr[:],
                        in0=mv[:, 0:1],
                        scalar1=rstd[:],
                        scalar2=-1.0,
                        op0=OP.mult,
                        op1=OP.mult,
                    )
                    z = work.tile([128, D], f32, tag="z", bufs=2, name=f"z{it}")
                    nc.vector.tensor_scalar(
                        out=z[:],
                        in0=y[:],
                        scalar1=rstd[:],
                        scalar2=nmr[:],
                        op0=OP.mult,
                        op1=OP.add,
                    )
                    if not trivial_ln:
                        z2 = work.tile([128, D], f32, tag="z2", bufs=2, name=f"z2{it}")
                        nc.vector.tensor_mul(out=z2[:], in0=z[:], in1=gbc_sb[:])
                        z3 = work.tile([128, D], f32, tag="z3", bufs=2, name=f"z3{it}")
                        nc.vector.tensor_add(out=z3[:], in0=z2[:], in1=bbc_sb[:])
                        z = z3
                    nc.sync.dma_start(out=out_d[it * 128 : (it + 1) * 128, :], in_=z[:])

    nc.compile()
    return nc


def _get_nc(trivial_ln: bool, debug: bool = False):
    key = ("nc", trivial_ln, debug)
    if key not in _CACHE:
        _CACHE[key] = _build(trivial_ln, debug)
    return _CACHE[key]


def _shard(inputs):
    import ml_dtypes

    bf = ml_dtypes.bfloat16
    q = np.ascontiguousarray(np.asarray(inputs["q"], dtype=np.float32))
    k = np.ascontiguousarray(np.asarray(inputs["k"], dtype=np.float32))
    v = np.ascontiguousarray(np.asarray(inputs["v"], dtype=np.float32))
    w_q = np.asarray(inputs["w_q"], dtype=np.float32)
    w_k = np.asarray(inputs["w_k"], dtype=np.float32)
    w_v = np.asarray(inputs["w_v"], dtype=np.float32)
    w_fc = np.asarray(inputs["w_fc"], dtype=np.float32)
    gamma = np.asarray(inputs["ln_gamma"], dtype=np.float32).reshape(1, D)
    beta = np.asarray(inputs["ln_beta"], dtype=np.float32).reshape(1, D)

    xt = {}
    for gi in range(2):
        xt[gi] = (
            np.ascontiguousarray(q[gi].T).astype(bf),
            np.ascontiguousarray(k[gi].T).astype(bf),
            np.ascontiguousarray(v[gi].T).astype(bf),
        )
    wq_s = [np.ascontiguousarray(w_q[:, p * DPC : (p + 1) * DPC]).astype(bf) for p in range(PG)]
    wk_s = [np.ascontiguousarray(w_k[:, p * DPC : (p + 1) * DPC]).astype(bf) for p in range(PG)]
    wv_s = [np.ascontiguousarray(w_v[:, p * DPC : (p + 1) * DPC]).astype(bf) for p in range(PG)]
    # extended fc weights: row block r (256 rows) = w_fc rows for rank r's heads,
    # zeroed when rank r belongs to the other batch group
    wfcx = []
    for gi in range(2):
        w = np.zeros((2 * D, D), dtype=np.float32)
        for r in range(NCORES):
            if r // PG == gi:
                hp = r % PG
                w[r * DPC : (r + 1) * DPC, :] = w_fc[hp * DPC : (hp + 1) * DPC, :]
        wfcx.append(w.astype(bf))

    in_maps = []
    for c in range(NCORES):
        gi, p = divmod(c, PG)
        in_maps.append(
            {
                "xt_q": xt[gi][0],
                "xt_k": xt[gi][1],
                "xt_v": xt[gi][2],
                "wq": wq_s[p],
                "wk": wk_s[p],
                "wv": wv_s[p],
                "wfcx": wfcx[gi],
                "resid": np.ascontiguousarray(q[gi, p * SL : (p + 1) * SL, :]),
                "gamma": gamma,
                "beta": beta,
            }
        )
    trivial_ln = bool(np.all(gamma == 1.0) and np.all(beta == 0.0))
    return in_maps, trivial_ln


def _run(inputs, trace=False, debug=False):
    from concourse.bass_utils import run_bass_kernel_spmd

    in_maps, trivial_ln = _shard(inputs)
    nc = _get_nc(trivial_ln, debug)
    res = run_bass_kernel_spmd(
        nc, in_maps, core_ids=list(range(NCORES)), trace=trace
    )
    out = np.empty((B, S, D), dtype=np.float32)
    for c in range(NCORES):
        gi, p = divmod(c, PG)
        out[gi, p * SL : (p + 1) * SL, :] = res.results[c]["out"]
    return out, res


def kernel(**inputs) -> np.ndarray:
    out, _ = _run(inputs)
    return out


def _timed_exec(inputs, iters=5):
    """Execute on 8 cores with device-resident inputs; return (out, [dt_ns])."""
    import time

    import jax
    import jax.numpy as jnp  # noqa: F401
    from jax.sharding import Mesh, PartitionSpec
    from jax.experimental.shard_map import shard_map

    import concourse.mybir as mybir
    from concourse import bass2jax

    in_maps, trivial_ln = _shard(inputs)
    nc = _get_nc(trivial_ln)
    bass2jax.install_neuronx_cc_hook()

    n_cores = NCORES
    partition_name = nc.partition_id_tensor.name if nc.partition_id_tensor else None
    in_names, out_names, out_avals, zero_outs = [], [], [], []
    for alloc in nc.m.functions[0].allocations:
        if not isinstance(alloc, mybir.MemoryLocationSet):
            continue
        name = alloc.memorylocations[0].name
        if alloc.kind == "ExternalInput":
            if name != partition_name:
                in_names.append(name)
        elif alloc.kind == "ExternalOutput":
            shape = tuple(alloc.tensor_shape)
            dtype = mybir.dt.np(alloc.dtype)
            out_names.append(name)
            out_avals.append(jax.core.ShapedArray(shape, dtype))
            zero_outs.append(np.zeros(shape, dtype))
    n_params = len(in_names)
    n_outs = len(out_avals)
    all_names = in_names + out_names
    if partition_name is not None:
        all_names = all_names + [partition_name]
    donate = tuple(range(n_params, n_params + n_outs))

    def _body(*args):
        operands = list(args)
        if partition_name is not None:
            operands.append(bass2jax.partition_id_tensor())
        outs = bass2jax._bass_exec_p.bind(
            *operands,
            out_avals=tuple(out_avals),
            in_names=tuple(all_names),
            out_names=tuple(out_names),
            lowering_input_output_aliases=(),
            sim_require_finite=True,
            sim_require_nnan=True,
            nc=nc,
        )
        return tuple(outs)

    devices = jax.devices()[:n_cores]
    mesh = Mesh(np.asarray(devices), ("core",))
    in_specs = (PartitionSpec("core"),) * (n_params + n_outs)
    out_specs = (PartitionSpec("core"),) * n_outs
    sharded = jax.jit(
        shard_map(_body, mesh=mesh, in_specs=in_specs, out_specs=out_specs, check_rep=False),
        donate_argnums=donate,
        keep_unused=True,
    )
    from jax.sharding import NamedSharding

    shd = NamedSharding(mesh, PartitionSpec("core"))
    concat_in = [
        jax.device_put(
            np.concatenate([np.asarray(in_maps[c][n]) for c in range(n_cores)], axis=0), shd
        )
        for n in in_names
    ]
    times = []
    out_arrs = None
    for _ in range(iters):
        zeros_dev = [
            jax.device_put(np.zeros((n_cores * z.shape[0], *z.shape[1:]), z.dtype), shd)
            for z in zero_outs
        ]
        jax.block_until_ready(zeros_dev)
        t0 = time.perf_counter()
        out_arrs = sharded(*concat_in, *zeros_dev)
        jax.block_until_ready(out_arrs)
        times.append((time.perf_counter() - t0) * 1e9)

    # pipelined: many dispatches in flight so the axon RPC overhead overlaps;
    # marginal per-call time approximates device execution time
    npipe = 16
    zsets = [
        [
            jax.device_put(np.zeros((n_cores * z.shape[0], *z.shape[1:]), z.dtype), shd)
            for z in zero_outs
        ]
        for _ in range(npipe)
    ]
    jax.block_until_ready(zsets)
    outs = [sharded(*concat_in, *zsets[0])]
    jax.block_until_ready(outs[0])  # warm
    t0 = time.perf_counter()
    for i in range(1, npipe):
        outs.append(sharded(*concat_in, *zsets[i]))
    jax.block_until_ready(outs)
    tp = (time.perf_counter() - t0) * 1e9 / (npipe - 1)
    times.append(tp)
    out = np.empty((B, S, D), dtype=np.float32)
    full = np.asarray(out_arrs[out_names.index("out")]).reshape(n_cores, SL, D)
    for c in range(n_cores):
        gi, p = divmod(c, PG)
        out[gi, p * SL : (p + 1) * SL, :] = full[c]
    return out, times


def _dispatch_floor(iters=5):
    """Measure the axon dispatch floor with a trivial jitted op on all 8 devices."""
    import time

    import jax
    import jax.numpy as jnp
    from jax.sharding import Mesh, PartitionSpec, NamedSharding

    devices = jax.devices()[:NCORES]
    mesh = Mesh(np.asarray(devices), ("core",))
    shd = NamedSharding(mesh, PartitionSpec("core"))
    x = jax.device_put(np.ones((NCORES, 8), np.float32), shd)
    f = jax.jit(lambda a: a + 1.0)
    jax.block_until_ready(f(x))
    times = []
    for _ in range(iters):
        t0 = time.perf_counter()
        jax.block_until_ready(f(x))
        times.append((time.perf_counter() - t0) * 1e9)
    return times
